# revision 61
# baseline (speedup 1.0000x reference)
"""GraphUNet (N=4096, E=65536, C=256, depth 3, ratio 0.5) on 8 trn2 NeuronCores.

Row-sharded SPMD pipeline, 6 launches. Device does the O(n^2 C) message
passing (N @ Z SpMMs) and the dense A@A augmentations; host does top-k,
gathers, O(nC^2) feature transforms / weight folds, and O(n^2)
element-wise adjacency prep between launches.

Layout/overlap notes: every DRAM tensor ships partition-major
([128, K/128, F]) so DMA descriptors move >=512B contiguous runs at the
full modeled 360 GB/s; input DMAs are emitted in consumption order with
the streamed operand chunk-interleaved so the PE starts ~4us in; output
DMAs are emitted last (the DMA queue is in-order, so a store's semaphore
wait would block later loads); diag/bias GCN corrections are folded into
the PSUM accumulation as rank-1 / diagonal matmul chains, leaving one
fused tensor_scalar per output block.

  K1   init GCN:   x0[sl] = dis0*(A0[sl] @ (zhi+zlo) + 2*dis0*y0[sl]
                   + (1/dis0) x b), A0 in exact small-int fp8 (DoubleRow),
                   z = dis0*(x@Wi) as an fp8 hi+lo split.
  K2-K4a levels:   M^T col-block = R^T @ L[sl]^T chain in fp8 (exact ints;
                   bf16 at level 3), shipped back raw (ints, fp8/bf16
                   exact off-diag; diag rounding host-compensated in the
                   in-chain diag correction); the down GCN reuses the
                   in-SBUF M^T block as lhsT against host-prescaled
                   yp = dis*(xp@Wd) (fp8 hi+lo DoubleRow at level 1),
                   with diag/bias corrections in-chain and relu fused
                   into the dis row-scale consumer.
  K4b  up GCNs:    xU1 = relu(N2 @ z2 + b) replicated (z2 host-folds the
                   deepest unpool scatter); the middle product is built
                   directly in transposed form (lhsT/rhs roles swapped)
                   so no on-device transpose is needed, with its xd1 part
                   running fp8 DoubleRow against raw-int M1 (diag := 2)
                   and the output-column dis scale deferred to the xU2
                   consumer; biases enter psums via rank-1 ones chains.
  K4c  final GCN:  identical program to K1 on zf = (x0 + scatter(perm0,
                   xU2)) @ Wf -- the unpool scatter and Wf fold on host
                   collapse the two chains of the reference into one.

Precision: adjacency chains exact; features bf16 (fp8 hi+lo where they
feed the big A-chains); measured end-to-end rel err ~1.1e-2 vs f32 ref.
"""

import numpy as np
import ml_dtypes

from contextlib import ExitStack

import concourse.bass as bass
import concourse.mybir as mybir
import concourse.tile as tile
from concourse import bacc
from concourse.bass_utils import run_bass_kernel_spmd

NCORES = 8
C = 256
F32 = mybir.dt.float32
F32R = mybir.dt.float32r
BF16 = mybir.dt.bfloat16
FP8 = mybir.dt.float8e4

NP_OF = {F32: np.float32, F32R: np.float32,
         BF16: ml_dtypes.bfloat16, FP8: ml_dtypes.float8_e4m3fn}

_TRACE = {"on": False, "results": [], "ncs": []}
DR = mybir.MatmulPerfMode.DoubleRow
MULT = mybir.AluOpType.mult
MAXOP = mybir.AluOpType.max


def _new_nc():
    return bacc.Bacc("TRN2", target_bir_lowering=False, debug=False,
                     num_devices=NCORES)


def _finish(nc):
    nc.compile()
    _TRACE["ncs"].append(nc)
    return nc


def _run(nc, in_maps):
    res = run_bass_kernel_spmd(nc, in_maps, list(range(NCORES)),
                               trace=_TRACE["on"])
    if _TRACE["on"]:
        _TRACE["results"].append(res)
    return res.results


def _ld(nc, t, dram, k0, k1, f0=None, f1=None):
    if f0 is None:
        nc.sync.dma_start(t[:, k0:k1, :], dram.ap()[:, k0:k1, :])
    else:
        nc.sync.dma_start(t[:, k0:k1, f0:f1], dram.ap()[:, k0:k1, f0:f1])


# ------------------------------------------------------------ K1 / K4c
def build_diag():
    """x[sl] = dis[sl]*((A0+2I)[sl] @ (zhi+zlo)) + (1/dis)xb scaled.
    [4096 -> 512/core].  The +2I diagonal folds into the fp8 adjacency
    (still exact small ints).

    AT ships mo-major so each 128-row output chain completes as soon as
    its A-slice lands; consumers and stores pipeline behind the PE."""
    n, rpc, KT, MO = 4096, 512, 32, 4
    nc = _new_nc()
    AT = nc.dram_tensor("AT", [128, MO, KT, 128], FP8,
                        kind="ExternalInput")
    ZH = nc.dram_tensor("ZH", [128, KT, C], FP8, kind="ExternalInput")
    ZL = nc.dram_tensor("ZL", [128, KT, C], FP8, kind="ExternalInput")
    DISP = nc.dram_tensor("DISP", [128, MO], F32, kind="ExternalInput")
    IVBV = nc.dram_tensor("IVBV", [1, rpc + C], BF16,
                          kind="ExternalInput")
    XO = nc.dram_tensor("XO", [128, MO, C], F32, kind="ExternalOutput")
    with tile.TileContext(nc) as tc:
        ctx = ExitStack()
        sb = ctx.enter_context(tc.tile_pool(name="sb", bufs=1))
        ps = ctx.enter_context(tc.tile_pool(name="ps", bufs=1, space="PSUM"))
        at_sb = sb.tile([128, MO, KT, 128], FP8, tag="at")
        zh_sb = sb.tile([128, KT, C], FP8, tag="zh")
        zl_sb = sb.tile([128, KT, C], FP8, tag="zl")
        disp_sb = sb.tile([128, MO], F32, tag="disp")
        ivbv_sb = sb.tile([128, rpc + C], BF16, tag="ivbv")
        nc.sync.dma_start(at_sb[:, 0, :, :], AT.ap()[:, 0, :, :])
        nc.sync.dma_start(disp_sb[:], DISP.ap())
        nc.sync.dma_start(ivbv_sb[:1, :], IVBV.ap())
        _ld(nc, zh_sb, ZH, 0, 8)
        _ld(nc, zl_sb, ZL, 0, 8)
        nc.sync.dma_start(at_sb[:, 1, :, :], AT.ap()[:, 1, :, :])
        _ld(nc, zh_sb, ZH, 8, 16)
        _ld(nc, zl_sb, ZL, 8, 16)
        nc.sync.dma_start(at_sb[:, 2, :, :], AT.ap()[:, 2, :, :])
        _ld(nc, zh_sb, ZH, 16, 24)
        _ld(nc, zl_sb, ZL, 16, 24)
        _ld(nc, zh_sb, ZH, 24, 32)
        _ld(nc, zl_sb, ZL, 24, 32)
        nc.sync.dma_start(at_sb[:, 3, :, :], AT.ap()[:, 3, :, :])
        xo_sb = sb.tile([128, MO, C], F32, tag="xo")
        pss = [ps.tile([128, C], F32, tag=f"p{m}", name=f"p{m}")
               for m in range(MO)]
        # p-state warmers: keep the PE continuously busy through the load
        # stream so the real chains run at the fully-ramped clock
        wps = ps.tile([128, 128], F32, tag="w", name="w")
        for _ in range(60):
            nc.tensor.matmul(wps[:], at_sb[:, 0, 0:2, :],
                             at_sb[:, 0, 0:2, :], start=True, stop=True,
                             perf_mode=DR)

        def chain(mo, kps, first, last):
            for kp in kps:
                for ci, ch in enumerate((zh_sb, zl_sb)):
                    nc.tensor.matmul(
                        pss[mo][:], at_sb[:, mo, 2 * kp:2 * kp + 2, :],
                        ch[:, 2 * kp:2 * kp + 2, :],
                        start=(first and kp == kps[0] and ci == 0),
                        stop=False, perf_mode=DR)
            if last:
                nc.tensor.matmul(pss[mo][:],
                                 ivbv_sb[:1, mo * 128:(mo + 1) * 128],
                                 ivbv_sb[:1, rpc:], start=False, stop=True)
                nc.any.tensor_scalar_mul(xo_sb[:, mo, :], pss[mo][:],
                                         disp_sb[:, mo:mo + 1])

        for mo in range(MO):
            chain(mo, list(range(16)), True, True)
        for mo in range(MO):
            nc.sync.dma_start(XO.ap()[:, mo, :], xo_sb[:, mo, :])
        ctx.close()
    return _finish(nc)


# ----------------------------------------------------- K2 / K3 / K4a
def build_level(npv, n, rpc, adt, ship, mt8=False):
    """M^T col-block = R^T @ L[sl]^T (exact ints), then
    x[sl] = relu(dis[sl] * (M[sl] @ yp + diag(2-diagM) yp[sl]
    + (1/dis) x b)).  Ships raw M^T if `ship`.  With mt8, the M block
    stays fp8 (offdiag ints <= 16 exact, diag host-compensated) and the
    GCN runs fp8 DoubleRow over an yp hi+lo split."""
    KTp, KT = npv // 128, n // 128
    mo2 = (rpc + 127) // 128
    mdt = FP8 if mt8 else BF16
    nc = _new_nc()
    R = nc.dram_tensor("R", [128, KTp, n], adt, kind="ExternalInput")
    LT = nc.dram_tensor("LT", [128, KTp, rpc], adt, kind="ExternalInput")
    if mt8:
        YPH = nc.dram_tensor("YPH", [128, KT, C], FP8,
                             kind="ExternalInput")
        YPL = nc.dram_tensor("YPL", [128, KT, C], FP8,
                             kind="ExternalInput")
    else:
        YP = nc.dram_tensor("YP", [128, KT, C], BF16,
                            kind="ExternalInput")
    DISP = nc.dram_tensor("DISP", [128, mo2], F32, kind="ExternalInput")
    # DGYS packs the diag-correction lhsT (128) and its ys rows (C)
    DGYS = nc.dram_tensor("DGYS", [128, mo2, 128 + C], BF16,
                          kind="ExternalInput")
    # IVBV packs the rank-1 bias chain: row0 = [1/dis[sl] | b]
    IVBV = nc.dram_tensor("IVBV", [1, max(rpc, 128) + C], BF16,
                          kind="ExternalInput")
    if ship:
        MT = nc.dram_tensor("MT", [128, KT, rpc], mdt,
                            kind="ExternalOutput")
    XO = nc.dram_tensor("XO", [128, mo2, C] if rpc >= 128 else [rpc, C],
                        F32, kind="ExternalOutput")
    # PSUM groups of <=4 chain banks (+ mo2 GCN banks), tail split into
    # 2-block groups so the last chain+copy+ship is short. R loads use
    # separate f-spans (unions of groups) so each DMA moves >=512B
    # contiguous runs.
    groups = []
    rem = list(range(KT))
    while rem:
        take = 4 if len(rem) > 4 else 2
        groups.append(rem[:take])
        rem = rem[take:]
    span_blocks = max(1, 512 // (128 * mybir.dt.size(adt)))
    load_spans = []
    cur = None
    for g in groups:
        cur = [g[0], g[-1] + 1] if cur is None else [cur[0], g[-1] + 1]
        if cur[1] - cur[0] >= span_blocks:
            load_spans.append(tuple(cur))
            cur = None
    if cur is not None:
        if load_spans:
            load_spans[-1] = (load_spans[-1][0], cur[1])
        else:
            load_spans.append(tuple(cur))
    with tile.TileContext(nc) as tc:
        ctx = ExitStack()
        sb = ctx.enter_context(tc.tile_pool(name="sb", bufs=1))
        ps = ctx.enter_context(tc.tile_pool(name="ps", bufs=1, space="PSUM"))
        disp_sb = sb.tile([128, mo2], F32, tag="disp")
        dgys_sb = sb.tile([128, mo2, 128 + C], BF16, tag="dgys")
        ivbv_sb = sb.tile([128, max(rpc, 128) + C], BF16, tag="ivbv")
        lt_sb = sb.tile([128, KTp, rpc], adt, tag="lt")
        r_sb = sb.tile([128, KTp, n], adt, tag="r")
        if mt8:
            yph_sb = sb.tile([128, KT, C], FP8, tag="yph")
            ypl_sb = sb.tile([128, KT, C], FP8, tag="ypl")
        else:
            yp_sb = sb.tile([128, KT, C], BF16, tag="yp")
        for si, (b0, b1) in enumerate(load_spans):
            f0, f1 = b0 * 128, b1 * 128
            if si == len(load_spans) - 1:
                # finer chunks at the stream tail: less work gated on
                # the final bytes
                kchunks = [(k, min(KTp, k + 8)) for k in
                           range(0, max(0, KTp - 8), 8)] + \
                    [(max(0, KTp - 8), max(0, KTp - 4)), (KTp - 4, KTp)]
            else:
                kchunks = [(k, min(KTp, k + 8)) for k in range(0, KTp, 8)]
            for k0, k1 in kchunks:
                if si == 0:
                    _ld(nc, lt_sb, LT, k0, k1)
                _ld(nc, r_sb, R, k0, k1, f0, f1)
            if si == 0:
                if mt8:
                    for k0 in range(0, KT, 8):
                        _ld(nc, yph_sb, YPH, k0, min(KT, k0 + 8))
                        _ld(nc, ypl_sb, YPL, k0, min(KT, k0 + 8))
                else:
                    for k0 in range(0, KT, 8):
                        _ld(nc, yp_sb, YP, k0, min(KT, k0 + 8))
        nc.sync.dma_start(disp_sb[:], DISP.ap())
        nc.sync.dma_start(dgys_sb[:], DGYS.ap())
        nc.sync.dma_start(ivbv_sb[:1, :], IVBV.ap())
        mt_sb = sb.tile([128, KT, rpc], mdt, tag="mt")
        xo_sb = sb.tile([128, mo2, C], F32, tag="xo")
        use_dr = adt == FP8
        gps = [ps.tile([128, C], F32, tag=f"g{m}", name=f"g{m}")
               for m in range(mo2)]
        msz2 = min(128, rpc)
        # p-state warmers (lowest-priority filler on LT data): keep the PE
        # ramped to full clock through the arrival-gated M-chain bursts.
        # With mo2 == 2 the banks go to dedicated second-to-last-group
        # psums instead (warmers measured no effect there).
        if mo2 < 2:
            wps = ps.tile([128, rpc], F32, tag="w", name="w")
            for _ in range(40 if KTp <= 16 else 80):
                if use_dr:
                    nc.tensor.matmul(wps[:msz2, :], lt_sb[:, 0:2, 0:msz2],
                                     lt_sb[:, 0:2, :], start=True,
                                     stop=True, perf_mode=DR)
                else:
                    nc.tensor.matmul(wps[:msz2, :], lt_sb[:, 0, 0:msz2],
                                     lt_sb[:, 0, :], start=True, stop=True)

        def ptag(gi, mo):
            # dedicated banks for the second-to-last group so its chains
            # need not wait on the previous group's consumers
            if mo2 == 2 and gi == len(groups) - 2:
                return f"q{mo % 2}"
            return f"p{mo % 4}"
        for gi, mos in enumerate(groups):
            pss = {m: ps.tile([128, rpc], F32, tag=ptag(gi, m),
                              name=f"p{m}") for m in mos}
            if use_dr:
                for kp in range(KTp // 2):
                    for mo in mos:
                        nc.tensor.matmul(
                            pss[mo][:],
                            r_sb[:, 2 * kp:2 * kp + 2,
                                 mo * 128:(mo + 1) * 128],
                            lt_sb[:, 2 * kp:2 * kp + 2, :],
                            start=(kp == 0), stop=(kp == KTp // 2 - 1),
                            perf_mode=DR)
            else:
                for kt in range(KTp):
                    for mo in mos:
                        nc.tensor.matmul(
                            pss[mo][:],
                            r_sb[:, kt, mo * 128:(mo + 1) * 128],
                            lt_sb[:, kt, :],
                            start=(kt == 0), stop=(kt == KTp - 1))
            for mo in mos:
                nc.any.tensor_copy(mt_sb[:, mo, :], pss[mo][:])
            lastg = gi == len(groups) - 1

            def gcn_mms(m, mos, stop_here):
                if mt8:
                    kpl = list(range(mos[0] // 2, (mos[-1] + 1) // 2))
                    for kp in kpl:
                        for ci, ch in enumerate((yph_sb, ypl_sb)):
                            nc.tensor.matmul(
                                gps[m][:msz2, :],
                                mt_sb[:, 2 * kp:2 * kp + 2,
                                      m * 128:m * 128 + msz2],
                                ch[:, 2 * kp:2 * kp + 2, :],
                                start=(kp == 0 and ci == 0),
                                stop=(stop_here and kp == kpl[-1]
                                      and ci == 1),
                                perf_mode=DR)
                else:
                    for kt in mos:
                        nc.tensor.matmul(
                            gps[m][:msz2, :],
                            mt_sb[:, kt, m * 128:m * 128 + msz2],
                            yp_sb[:, kt, :], start=(kt == 0),
                            stop=(stop_here and kt == mos[-1]))

            for m in range(mo2):
                gcn_mms(m, mos, lastg)
                if lastg:
                    nc.vector.tensor_scalar(
                        xo_sb[:msz2, m, :], gps[m][:msz2, :],
                        disp_sb[:msz2, m:m + 1], 0.0, MULT, MAXOP)
            if gi == 0:
                # diag/bias corrections ride mid-chain (data ready early)
                W = max(rpc, 128)
                for m in range(mo2):
                    nc.tensor.matmul(gps[m][:msz2, :],
                                     dgys_sb[:msz2, m, :msz2],
                                     dgys_sb[:msz2, m, 128:],
                                     start=False, stop=False)
                    nc.tensor.matmul(gps[m][:msz2, :],
                                     ivbv_sb[:1, m * 128:m * 128 + msz2],
                                     ivbv_sb[:1, W:], start=False,
                                     stop=False)
        if ship:
            for gi, mos in enumerate(groups):
                nc.sync.dma_start(MT.ap()[:, mos[0]:mos[-1] + 1, :],
                                  mt_sb[:, mos[0]:mos[-1] + 1, :])
        if rpc >= 128:
            nc.sync.dma_start(XO.ap(), xo_sb[:])
        else:
            nc.sync.dma_start(XO.ap(), xo_sb[:rpc, 0, :])
        ctx.close()
    return _finish(nc)


# ------------------------------------------------------------------ K4b
def build_k4b():
    """xU1 = relu(N2 @ z2 + b0) replicated;
    xU2[sl] = relu((N1[sl]@xd1 + N1[sl][:,perm1]@xU1) @ Wu1 + b1).

    The middle product is computed directly in transposed form with the
    output-column dis scale deferred to the xU2 consumer:
      v2T' = (dis*xd1)^T-split-fp8 @ M1R (raw ints, diag:=2, DoubleRow)
           + xU1^T-chain @ Q1B' (dis-col-prescaled, bf16),
    so no on-device transpose is needed, the big xd1 chain runs at fp8
    DoubleRow rate, and v2T' feeds the Wu1 matmul as lhsT directly."""
    nc = _new_nc()
    NT2 = nc.dram_tensor("NT2", [128, 8, 1024], BF16, kind="ExternalInput")
    Z2 = nc.dram_tensor("Z2", [128, 8, C], BF16, kind="ExternalInput")
    XDH = nc.dram_tensor("XDH", [128, 16, C], FP8, kind="ExternalInput")
    XDL = nc.dram_tensor("XDL", [128, 16, C], FP8, kind="ExternalInput")
    NT1R = nc.dram_tensor("NT1R", [128, 16, C], FP8, kind="ExternalInput")
    Q1B = nc.dram_tensor("Q1B", [128, 8, C], BF16, kind="ExternalInput")
    WU1 = nc.dram_tensor("WU1", [128, 2, C], BF16, kind="ExternalInput")
    ONES = nc.dram_tensor("ONES", [1, 128], BF16, kind="ExternalInput")
    B0 = nc.dram_tensor("B0", [1, C], BF16, kind="ExternalInput")
    DISPQ = nc.dram_tensor("DISPQ", [128, 2], F32, kind="ExternalInput")
    IVB1 = nc.dram_tensor("IVB1", [1, C + C], BF16, kind="ExternalInput")
    XO = nc.dram_tensor("XO", [128, 2, C], F32, kind="ExternalOutput")
    with tile.TileContext(nc) as tc:
        ctx = ExitStack()
        sb = ctx.enter_context(tc.tile_pool(name="sb", bufs=1))
        ps = ctx.enter_context(tc.tile_pool(name="ps", bufs=1, space="PSUM"))
        xdh_sb = sb.tile([128, 16, C], FP8, tag="xdh")
        xdl_sb = sb.tile([128, 16, C], FP8, tag="xdl")
        nt1r_sb = sb.tile([128, 16, C], FP8, tag="nt1r")
        for k0 in range(0, 16, 8):
            _ld(nc, xdh_sb, XDH, k0, k0 + 8)
            _ld(nc, xdl_sb, XDL, k0, k0 + 8)
            _ld(nc, nt1r_sb, NT1R, k0, k0 + 8)
        z2_sb = sb.tile([128, 8, C], BF16, tag="z2")
        nc.sync.dma_start(z2_sb[:], Z2.ap())
        nt2_sb = sb.tile([128, 8, 1024], BF16, tag="nt2")
        for k0 in range(0, 8, 2):
            _ld(nc, nt2_sb, NT2, k0, k0 + 2)
        q1b_sb = sb.tile([128, 8, C], BF16, tag="q1b")
        nc.sync.dma_start(q1b_sb[:], Q1B.ap())
        ones_sb = sb.tile([128, 128], BF16, tag="ones")
        nc.sync.dma_start(ones_sb[:1, :], ONES.ap())
        b0_sb = sb.tile([128, C], BF16, tag="b0")
        nc.sync.dma_start(b0_sb[:1, :], B0.ap())
        dispq_sb = sb.tile([128, 2], F32, tag="dispq")
        nc.sync.dma_start(dispq_sb[:], DISPQ.ap())
        ivb1_sb = sb.tile([128, C + C], BF16, tag="ivb1")
        nc.sync.dma_start(ivb1_sb[:1, :], IVB1.ap())
        wu1_sb = sb.tile([128, 2, C], BF16, tag="wu1")
        nc.sync.dma_start(wu1_sb[:], WU1.ap())

        xu1_sb = sb.tile([128, 8, C], BF16, tag="xu1")
        pv = [ps.tile([128, C], F32, tag=f"v{m}", name=f"v{m}")
              for m in range(2)]
        # xd1 part: fp8 DoubleRow split against raw-int M1 (diag:=2)
        for kp in range(8):
            for ci, ch in enumerate((xdh_sb, xdl_sb)):
                for cm in range(2):
                    nc.tensor.matmul(
                        pv[cm][:],
                        ch[:, 2 * kp:2 * kp + 2,
                           cm * 128:(cm + 1) * 128],
                        nt1r_sb[:, 2 * kp:2 * kp + 2, :],
                        start=(kp == 0 and ci == 0), stop=False,
                        perf_mode=DR)
        xu1_groups = [list(range(6)), [6, 7]]
        for mos in xu1_groups:
            pss = {m: ps.tile([128, C], F32, tag=f"p{m % 6}",
                              name=f"pu{m}") for m in mos}
            for kt in range(8):
                for mo in mos:
                    nc.tensor.matmul(
                        pss[mo][:], nt2_sb[:, kt, mo * 128:(mo + 1) * 128],
                        z2_sb[:, kt, :], start=(kt == 0), stop=False)
            for mo in mos:
                nc.tensor.matmul(pss[mo][:], ones_sb[:1, :], b0_sb[:1, :],
                                 start=False, stop=True)
                nc.vector.tensor_scalar_max(xu1_sb[:, mo, :], pss[mo][:],
                                            0.0)
                for cm in range(2):
                    nc.tensor.matmul(
                        pv[cm][:], xu1_sb[:, mo, cm * 128:(cm + 1) * 128],
                        q1b_sb[:, mo, :], start=False, stop=(mo == 7))
        v2t_sb = sb.tile([128, 2, C], BF16, tag="v2t")
        for cm in range(2):
            nc.any.tensor_copy(v2t_sb[:, cm, :], pv[cm][:])
        xo_sb = sb.tile([128, 2, C], F32, tag="xo")
        for mo in range(2):
            ps3 = ps.tile([128, C], F32, tag=f"p{mo + 2}", name="pw")
            for kt in range(2):
                nc.tensor.matmul(
                    ps3[:], v2t_sb[:, kt, mo * 128:(mo + 1) * 128],
                    wu1_sb[:, kt, :], start=(kt == 0), stop=False)
            nc.tensor.matmul(ps3[:], ivb1_sb[:1, mo * 128:(mo + 1) * 128],
                             ivb1_sb[:1, C:], start=False, stop=True)
            nc.vector.tensor_scalar(xo_sb[:, mo, :], ps3[:],
                                    dispq_sb[:, mo:mo + 1], 0.0,
                                    MULT, MAXOP)
        nc.sync.dma_start(XO.ap(), xo_sb[:])
        ctx.close()
    return _finish(nc)


# =================================================================== host
F8NP = ml_dtypes.float8_e4m3fn
BFNP = ml_dtypes.bfloat16


def _pm(a, dt):
    """[K, F] row-major -> partition-major [128, K//128, F]."""
    K, F = a.shape
    return np.ascontiguousarray(
        a.reshape(K // 128, 128, F).transpose(1, 0, 2)).astype(dt)


def _unpm(b):
    """[128, KT, F] -> [K, F]."""
    p, kt, f = b.shape
    return np.asarray(b, np.float32).transpose(1, 0, 2).reshape(kt * p, f)


def _pmv(v):
    """[K] -> [128, K//128] partition-major (padded to 128 rows)."""
    k = v.shape[0]
    if k < 128:
        v = np.pad(v, (0, 128 - k))
        k = 128
    return np.ascontiguousarray(
        v.reshape(k // 128, 128).T).astype(np.float32)


def _dgys(c, ys):
    """Pack diag-correction lhsT (c on the diagonal) with its ys rows:
    [128, mo2, 128 + C] bf16."""
    rpc = c.shape[0]
    mo2 = (rpc + 127) // 128
    out = np.zeros((128, mo2, 128 + ys.shape[1]), np.float32)
    for m in range(mo2):
        seg = c[m * 128:(m + 1) * 128]
        out[np.arange(len(seg)), m, np.arange(len(seg))] = seg
        rows = ys[m * 128:(m + 1) * 128]
        out[:rows.shape[0], m, 128:] = rows
    return out.astype(BFNP)


def _ivbv(iv, b, width):
    """Pack the rank-1 bias chain row: [1, width + C] bf16."""
    out = np.zeros((1, width + b.shape[0]), np.float32)
    out[0, :iv.shape[0]] = iv
    out[0, width:] = b
    return out.astype(BFNP)


def _mk_dis(deg):
    return (1.0 / np.sqrt(np.maximum(deg, 1e-12))).astype(np.float32)


def _diag_inputs(A8T, z, dis, bvec, rpc):
    """Per-core in_maps for the K1/K4c program. A8T is [4096, 4096] fp8
    (= (A0+2I)^T); the per-core AT block ships mo-major
    [128, MO, KT, 128]."""
    zhi = z.astype(F8NP)
    zlo = (z - zhi.astype(np.float32)).astype(F8NP)
    zhi_pm, zlo_pm = _pm(zhi, F8NP), _pm(zlo, F8NP)
    n = A8T.shape[0]
    maps = []
    for c in range(NCORES):
        sl = slice(c * rpc, (c + 1) * rpc)
        blk = A8T[:, sl]                       # [n, rpc]
        at = np.ascontiguousarray(
            blk.reshape(n // 128, 128, rpc // 128, 128)
            .transpose(1, 2, 0, 3))            # [128, MO, KT, 128]
        maps.append({
            "AT": at,
            "ZH": zhi_pm, "ZL": zlo_pm,
            "DISP": _pmv(dis[sl]),
            "IVBV": _ivbv(1.0 / dis[sl], np.asarray(bvec, np.float32),
                          rpc),
            })
    return maps


def kernel(x, edge_index, W_init, b_init, W_down, b_down, p_pool,
           W_up, b_up, W_final, b_final):
    x = np.asarray(x, np.float32)
    N = x.shape[0]
    rpc0 = N // NCORES

    A0 = np.zeros((N, N), np.float32)
    np.add.at(A0, (np.asarray(edge_index[0]), np.asarray(edge_index[1])),
              1.0)
    dis0 = _mk_dis(A0.sum(1) + 2.0)
    y0 = x @ np.asarray(W_init, np.float32)

    # exact level-0 score via host matvec (init GCN is linear)
    p0 = np.asarray(p_pool[0], np.float32)
    u = y0 @ p0
    s0 = (dis0 * (A0 @ (dis0 * u)) + 2.0 * dis0 * dis0 * u) \
        / np.linalg.norm(p0)
    perm0 = np.argsort(-s0, kind="stable")[:N // 2]
    sv0 = s0[perm0]

    # ---- K1
    A2I = np.ascontiguousarray(A0.T)               # (A0+2I)^T
    A2I[np.arange(N), np.arange(N)] += 2.0
    A8T = A2I.astype(F8NP)                         # [4096, 4096]
    nc1 = build_diag()
    maps = _diag_inputs(A8T, dis0[:, None] * y0, dis0,
                        np.asarray(b_init, np.float32), rpc0)
    res = _run(nc1, maps)
    x0 = np.concatenate([_unpm(r["XO"]) for r in res], 0)

    # ---- down levels
    Bh = A0 + np.eye(N, dtype=np.float32)
    xcur, perm, sv = x0, perm0, sv0
    n = N
    Ms, dis_l, xs, perms = [], [dis0], [x0], []
    level_fp8 = [True, True, False]
    for lev in range(3):
        k = n // 2
        rpc = k // NCORES
        perms.append(perm)
        L = Bh[perm, :]
        R = Bh[:, perm]
        lim = 16 if level_fp8[lev] else 256
        assert Bh.max() <= lim, (lev, Bh.max())
        diagM = np.einsum('it,ti->i', L, R, optimize=True)
        deg = L @ R.sum(1) - diagM + 2.0
        dis = _mk_dis(deg)
        xp = xcur[perm] * np.tanh(sv)[:, None]
        y = xp @ np.asarray(W_down[lev], np.float32)
        adt = FP8 if level_fp8[lev] else BF16
        npdt = NP_OF[adt]
        ship = lev < 2
        mt8 = lev == 0
        nc = build_level(n, k, rpc, adt, ship, mt8)
        Rpm = _pm(R, npdt)
        yfull = (dis[:, None] * y).astype(np.float32)
        bvec = np.asarray(b_down[lev], np.float32)
        # compensate the low-precision rounding of the in-SBUF M diagonal
        diag_b = diagM.astype(F8NP if mt8 else BFNP).astype(np.float32)
        if mt8:
            yph = yfull.astype(F8NP)
            ypl = (yfull - yph.astype(np.float32)).astype(F8NP)
            ypmaps = {"YPH": _pm(yph, F8NP), "YPL": _pm(ypl, F8NP)}
        else:
            ypmaps = {"YP": _pm(yfull, BFNP)}
        maps = []
        for cc in range(NCORES):
            sl = slice(cc * rpc, (cc + 1) * rpc)
            maps.append({
                "R": Rpm,
                "LT": _pm(np.ascontiguousarray(L[sl].T), npdt),
                **ypmaps,
                "DISP": _pmv(dis[sl]),
                "DGYS": _dgys(2.0 - diag_b[sl], yfull[sl]),
                "IVBV": _ivbv(1.0 / dis[sl], bvec, max(rpc, 128)),
                })
        res = _run(nc, maps)
        if rpc >= 128:
            xn = np.concatenate([_unpm(r["XO"]) for r in res], 0)
        else:
            xn = np.concatenate([np.asarray(r["XO"], np.float32)
                                 for r in res], 0)
        if ship:
            M = np.concatenate([_unpm(r["MT"]).T for r in res], 0)
            Ms.append(M)
            Bh = M.copy()
            np.fill_diagonal(Bh, 1.0)
        dis_l.append(dis)
        xs.append(xn)
        xcur, n = xn, k
        if lev < 2:
            pl = np.asarray(p_pool[lev + 1], np.float32)
            s = xn @ pl / np.linalg.norm(pl)
            perm = np.argsort(-s, kind="stable")[:k // 2]
            sv = s[perm]

    x_d1, x_d2, x_d3 = xs[1], xs[2], xs[3]
    dis1, dis2 = dis_l[1], dis_l[2]
    M1, M2 = Ms[0], Ms[1]
    perm1, perm2 = perms[1], perms[2]

    # ---- K4b
    N2 = M2.copy()
    np.fill_diagonal(N2, 2.0)
    N2 *= dis2[:, None] * dis2[None, :]
    N1 = M1.copy()
    np.fill_diagonal(N1, 2.0)
    N1 *= dis1[:, None] * dis1[None, :]
    up = np.zeros_like(x_d2)
    up[perm2] = x_d3
    z2 = (x_d2 + up) @ np.asarray(W_up[0], np.float32)
    nc4b = build_k4b()
    rpc1 = 2048 // NCORES
    nt2_pm = _pm(np.ascontiguousarray(N2.T), BFNP)
    z2_pm = _pm(z2, BFNP)
    xds = dis1[:, None] * x_d1
    xdh = xds.astype(F8NP)
    xdl = (xds - xdh.astype(np.float32)).astype(F8NP)
    xdh_pm, xdl_pm = _pm(xdh, F8NP), _pm(xdl, F8NP)
    wu1_pm = _pm(np.asarray(W_up[1], np.float32), BFNP)
    ones = np.ones((1, 128), BFNP)
    b0 = np.asarray(b_up[0], np.float32)[None, :].astype(BFNP)
    b1v = np.asarray(b_up[1], np.float32)
    maps = []
    for cc in range(NCORES):
        sl = slice(cc * rpc1, (cc + 1) * rpc1)
        nt1r = np.ascontiguousarray(M1[sl].T)      # raw ints, diag := 2
        nt1r[cc * rpc1 + np.arange(rpc1), np.arange(rpc1)] = 2.0
        maps.append({
            "NT2": nt2_pm, "Z2": z2_pm,
            "XDH": xdh_pm, "XDL": xdl_pm,
            "NT1R": _pm(nt1r, F8NP),
            "Q1B": _pm(np.ascontiguousarray(
                (N1[sl][:, perm1] / dis1[sl][:, None]).T), BFNP),
            "WU1": wu1_pm, "ONES": ones, "B0": b0,
            "DISPQ": _pmv(dis1[sl]),
            "IVB1": _ivbv(1.0 / dis1[sl], b1v, C),
            })
    res = _run(nc4b, maps)
    xU2 = np.concatenate([_unpm(r["XO"]) for r in res], 0)

    # ---- K4c
    upf = np.zeros_like(x0)
    upf[perm0] = xU2
    zf = (x0 + upf) @ np.asarray(W_final, np.float32)
    nc4c = build_diag()
    maps = _diag_inputs(A8T, dis0[:, None] * zf, dis0,
                        np.asarray(b_final, np.float32), rpc0)
    res = _run(nc4c, maps)
    out = np.concatenate([_unpm(r["XO"]) for r in res], 0)
    return out.astype(np.float32)


# revision 62
# speedup vs baseline: 1.0022x; 1.0022x over previous
"""GraphUNet (N=4096, E=65536, C=256, depth 3, ratio 0.5) on 8 trn2 NeuronCores.

Row-sharded SPMD pipeline, 6 launches. Device does the O(n^2 C) message
passing (N @ Z SpMMs) and the dense A@A augmentations; host does top-k,
gathers, O(nC^2) feature transforms / weight folds, and O(n^2)
element-wise adjacency prep between launches.

Layout/overlap notes: every DRAM tensor ships partition-major
([128, K/128, F]) so DMA descriptors move >=512B contiguous runs at the
full modeled 360 GB/s; input DMAs are emitted in consumption order with
the streamed operand chunk-interleaved so the PE starts ~4us in; output
DMAs are emitted last (the DMA queue is in-order, so a store's semaphore
wait would block later loads); diag/bias GCN corrections are folded into
the PSUM accumulation as rank-1 / diagonal matmul chains, leaving one
fused tensor_scalar per output block.

  K1   init GCN:   x0[sl] = dis0*(A0[sl] @ (zhi+zlo) + 2*dis0*y0[sl]
                   + (1/dis0) x b), A0 in exact small-int fp8 (DoubleRow),
                   z = dis0*(x@Wi) as an fp8 hi+lo split.
  K2-K4a levels:   M^T col-block = R^T @ L[sl]^T chain in fp8 (exact ints;
                   bf16 at level 3), shipped back raw (ints, fp8/bf16
                   exact off-diag; diag rounding host-compensated in the
                   in-chain diag correction); the down GCN reuses the
                   in-SBUF M^T block as lhsT against host-prescaled
                   yp = dis*(xp@Wd) (fp8 hi+lo DoubleRow at level 1),
                   with diag/bias corrections in-chain and relu fused
                   into the dis row-scale consumer.
  K4b  up GCNs:    xU1 = relu(N2 @ z2 + b) replicated (z2 host-folds the
                   deepest unpool scatter); the middle product is built
                   directly in transposed form (lhsT/rhs roles swapped)
                   so no on-device transpose is needed, with its xd1 part
                   running fp8 DoubleRow against raw-int M1 (diag := 2)
                   and the output-column dis scale deferred to the xU2
                   consumer; biases enter psums via rank-1 ones chains.
  K4c  final GCN:  identical program to K1 on zf = (x0 + scatter(perm0,
                   xU2)) @ Wf -- the unpool scatter and Wf fold on host
                   collapse the two chains of the reference into one.

Precision: adjacency chains exact; features bf16 (fp8 hi+lo where they
feed the big A-chains); measured end-to-end rel err ~1.1e-2 vs f32 ref.
"""

import numpy as np
import ml_dtypes

from contextlib import ExitStack

import concourse.bass as bass
import concourse.mybir as mybir
import concourse.tile as tile
from concourse import bacc
from concourse.bass_utils import run_bass_kernel_spmd

NCORES = 8
C = 256
F32 = mybir.dt.float32
F32R = mybir.dt.float32r
BF16 = mybir.dt.bfloat16
FP8 = mybir.dt.float8e4

NP_OF = {F32: np.float32, F32R: np.float32,
         BF16: ml_dtypes.bfloat16, FP8: ml_dtypes.float8_e4m3fn}

_TRACE = {"on": False, "results": [], "ncs": []}
DR = mybir.MatmulPerfMode.DoubleRow
MULT = mybir.AluOpType.mult
MAXOP = mybir.AluOpType.max


def _new_nc():
    return bacc.Bacc("TRN2", target_bir_lowering=False, debug=False,
                     num_devices=NCORES)


def _finish(nc):
    nc.compile()
    _TRACE["ncs"].append(nc)
    return nc


def _run(nc, in_maps):
    res = run_bass_kernel_spmd(nc, in_maps, list(range(NCORES)),
                               trace=_TRACE["on"])
    if _TRACE["on"]:
        _TRACE["results"].append(res)
    return res.results


def _ld(nc, t, dram, k0, k1, f0=None, f1=None):
    if f0 is None:
        nc.sync.dma_start(t[:, k0:k1, :], dram.ap()[:, k0:k1, :])
    else:
        nc.sync.dma_start(t[:, k0:k1, f0:f1], dram.ap()[:, k0:k1, f0:f1])


# ------------------------------------------------------------ K1 / K4c
def build_diag():
    """x[sl] = dis[sl]*((A0+2I)[sl] @ (zhi+zlo)) + (1/dis)xb scaled.
    [4096 -> 512/core].  The +2I diagonal folds into the fp8 adjacency
    (still exact small ints).

    AT ships mo-major so each 128-row output chain completes as soon as
    its A-slice lands; consumers and stores pipeline behind the PE."""
    n, rpc, KT, MO = 4096, 512, 32, 4
    nc = _new_nc()
    AT = nc.dram_tensor("AT", [128, MO, KT, 128], FP8,
                        kind="ExternalInput")
    ZH = nc.dram_tensor("ZH", [128, KT, C], FP8, kind="ExternalInput")
    ZL = nc.dram_tensor("ZL", [128, KT, C], FP8, kind="ExternalInput")
    DISP = nc.dram_tensor("DISP", [128, MO], F32, kind="ExternalInput")
    IVBV = nc.dram_tensor("IVBV", [1, rpc + C], BF16,
                          kind="ExternalInput")
    XO = nc.dram_tensor("XO", [128, MO, C], F32, kind="ExternalOutput")
    with tile.TileContext(nc) as tc:
        ctx = ExitStack()
        sb = ctx.enter_context(tc.tile_pool(name="sb", bufs=1))
        ps = ctx.enter_context(tc.tile_pool(name="ps", bufs=1, space="PSUM"))
        at_sb = sb.tile([128, MO, KT, 128], FP8, tag="at")
        zh_sb = sb.tile([128, KT, C], FP8, tag="zh")
        zl_sb = sb.tile([128, KT, C], FP8, tag="zl")
        disp_sb = sb.tile([128, MO], F32, tag="disp")
        ivbv_sb = sb.tile([128, rpc + C], BF16, tag="ivbv")
        nc.sync.dma_start(at_sb[:, 0, :, :], AT.ap()[:, 0, :, :])
        nc.sync.dma_start(disp_sb[:], DISP.ap())
        nc.sync.dma_start(ivbv_sb[:1, :], IVBV.ap())
        _ld(nc, zh_sb, ZH, 0, 8)
        _ld(nc, zl_sb, ZL, 0, 8)
        nc.sync.dma_start(at_sb[:, 1, :, :], AT.ap()[:, 1, :, :])
        _ld(nc, zh_sb, ZH, 8, 16)
        _ld(nc, zl_sb, ZL, 8, 16)
        nc.sync.dma_start(at_sb[:, 2, :, :], AT.ap()[:, 2, :, :])
        _ld(nc, zh_sb, ZH, 16, 24)
        _ld(nc, zl_sb, ZL, 16, 24)
        _ld(nc, zh_sb, ZH, 24, 32)
        _ld(nc, zl_sb, ZL, 24, 32)
        nc.sync.dma_start(at_sb[:, 3, :, :], AT.ap()[:, 3, :, :])
        xo_sb = sb.tile([128, MO, C], F32, tag="xo")
        pss = [ps.tile([128, C], F32, tag=f"p{m}", name=f"p{m}")
               for m in range(MO)]
        # p-state warmers: keep the PE continuously busy through the load
        # stream so the real chains run at the fully-ramped clock
        wps = ps.tile([128, 128], F32, tag="w", name="w")
        for _ in range(60):
            nc.tensor.matmul(wps[:], at_sb[:, 0, 0:2, :],
                             at_sb[:, 0, 0:2, :], start=True, stop=True,
                             perf_mode=DR)

        def chain(mo, kps, first, last):
            for kp in kps:
                for ci, ch in enumerate((zh_sb, zl_sb)):
                    nc.tensor.matmul(
                        pss[mo][:], at_sb[:, mo, 2 * kp:2 * kp + 2, :],
                        ch[:, 2 * kp:2 * kp + 2, :],
                        start=(first and kp == kps[0] and ci == 0),
                        stop=False, perf_mode=DR)
            if last:
                nc.tensor.matmul(pss[mo][:],
                                 ivbv_sb[:1, mo * 128:(mo + 1) * 128],
                                 ivbv_sb[:1, rpc:], start=False, stop=True)
                nc.any.tensor_scalar_mul(xo_sb[:, mo, :], pss[mo][:],
                                         disp_sb[:, mo:mo + 1])

        for mo in range(MO):
            chain(mo, list(range(16)), True, True)
        for mo in range(MO):
            nc.sync.dma_start(XO.ap()[:, mo, :], xo_sb[:, mo, :])
        ctx.close()
    return _finish(nc)


# ----------------------------------------------------- K2 / K3 / K4a
def build_level(npv, n, rpc, adt, ship, mt8=False):
    """M^T col-block = R^T @ L[sl]^T (exact ints), then
    x[sl] = relu(dis[sl] * (M[sl] @ yp + diag(2-diagM) yp[sl]
    + (1/dis) x b)).  Ships raw M^T if `ship`.  With mt8, the M block
    stays fp8 (offdiag ints <= 16 exact, diag host-compensated) and the
    GCN runs fp8 DoubleRow over an yp hi+lo split."""
    KTp, KT = npv // 128, n // 128
    mo2 = (rpc + 127) // 128
    mdt = FP8 if mt8 else BF16
    nc = _new_nc()
    R = nc.dram_tensor("R", [128, KTp, n], adt, kind="ExternalInput")
    LT = nc.dram_tensor("LT", [128, KTp, rpc], adt, kind="ExternalInput")
    if mt8:
        YPH = nc.dram_tensor("YPH", [128, KT, C], FP8,
                             kind="ExternalInput")
        YPL = nc.dram_tensor("YPL", [128, KT, C], FP8,
                             kind="ExternalInput")
    else:
        YP = nc.dram_tensor("YP", [128, KT, C], BF16,
                            kind="ExternalInput")
    DISP = nc.dram_tensor("DISP", [128, mo2], F32, kind="ExternalInput")
    # DGYS packs the diag-correction lhsT (128) and its ys rows (C)
    DGYS = nc.dram_tensor("DGYS", [128, mo2, 128 + C], BF16,
                          kind="ExternalInput")
    # IVBV packs the rank-1 bias chain: row0 = [1/dis[sl] | b]
    IVBV = nc.dram_tensor("IVBV", [1, max(rpc, 128) + C], BF16,
                          kind="ExternalInput")
    if ship:
        MT = nc.dram_tensor("MT", [128, KT, rpc], mdt,
                            kind="ExternalOutput")
    XO = nc.dram_tensor("XO", [128, mo2, C] if rpc >= 128 else [rpc, C],
                        F32, kind="ExternalOutput")
    # PSUM groups of <=4 chain banks (+ mo2 GCN banks), tail split into
    # 2-block groups so the last chain+copy+ship is short. R loads use
    # separate f-spans (unions of groups) so each DMA moves >=512B
    # contiguous runs.
    groups = []
    rem = list(range(KT))
    while rem:
        take = 4 if len(rem) > 4 else 2
        groups.append(rem[:take])
        rem = rem[take:]
    span_blocks = max(1, 512 // (128 * mybir.dt.size(adt)))
    load_spans = []
    cur = None
    for g in groups:
        cur = [g[0], g[-1] + 1] if cur is None else [cur[0], g[-1] + 1]
        if cur[1] - cur[0] >= span_blocks:
            load_spans.append(tuple(cur))
            cur = None
    if cur is not None:
        if load_spans:
            load_spans[-1] = (load_spans[-1][0], cur[1])
        else:
            load_spans.append(tuple(cur))
    with tile.TileContext(nc) as tc:
        ctx = ExitStack()
        sb = ctx.enter_context(tc.tile_pool(name="sb", bufs=1))
        ps = ctx.enter_context(tc.tile_pool(name="ps", bufs=1, space="PSUM"))
        disp_sb = sb.tile([128, mo2], F32, tag="disp")
        dgys_sb = sb.tile([128, mo2, 128 + C], BF16, tag="dgys")
        ivbv_sb = sb.tile([128, max(rpc, 128) + C], BF16, tag="ivbv")
        lt_sb = sb.tile([128, KTp, rpc], adt, tag="lt")
        r_sb = sb.tile([128, KTp, n], adt, tag="r")
        if mt8:
            yph_sb = sb.tile([128, KT, C], FP8, tag="yph")
            ypl_sb = sb.tile([128, KT, C], FP8, tag="ypl")
        else:
            yp_sb = sb.tile([128, KT, C], BF16, tag="yp")
        for si, (b0, b1) in enumerate(load_spans):
            f0, f1 = b0 * 128, b1 * 128
            if si == len(load_spans) - 1:
                # finer chunks at the stream tail: less work gated on
                # the final bytes
                kchunks = [(k, min(KTp, k + 8)) for k in
                           range(0, max(0, KTp - 8), 8)] + \
                    [(max(0, KTp - 8), max(0, KTp - 4)), (KTp - 4, KTp)]
            else:
                kchunks = [(k, min(KTp, k + 8)) for k in range(0, KTp, 8)]
            for k0, k1 in kchunks:
                if si == 0:
                    _ld(nc, lt_sb, LT, k0, k1)
                _ld(nc, r_sb, R, k0, k1, f0, f1)
            if si == 0:
                if mt8:
                    for k0 in range(0, KT, 8):
                        _ld(nc, yph_sb, YPH, k0, min(KT, k0 + 8))
                        _ld(nc, ypl_sb, YPL, k0, min(KT, k0 + 8))
                else:
                    for k0 in range(0, KT, 8):
                        _ld(nc, yp_sb, YP, k0, min(KT, k0 + 8))
        nc.sync.dma_start(disp_sb[:], DISP.ap())
        nc.sync.dma_start(dgys_sb[:], DGYS.ap())
        nc.sync.dma_start(ivbv_sb[:1, :], IVBV.ap())
        mt_sb = sb.tile([128, KT, rpc], mdt, tag="mt")
        xo_sb = sb.tile([128, mo2, C], F32, tag="xo")
        use_dr = adt == FP8
        gps = [ps.tile([128, C], F32, tag=f"g{m}", name=f"g{m}")
               for m in range(mo2)]
        msz2 = min(128, rpc)
        # p-state warmers (lowest-priority filler on LT data): keep the PE
        # ramped to full clock through the arrival-gated M-chain bursts.
        # With mo2 == 2 the banks go to dedicated second-to-last-group
        # psums instead (warmers measured no effect there).
        if mo2 < 2:
            wps = ps.tile([128, rpc], F32, tag="w", name="w")
            for _ in range(40 if KTp <= 16 else 80):
                if use_dr:
                    nc.tensor.matmul(wps[:msz2, :], lt_sb[:, 0:2, 0:msz2],
                                     lt_sb[:, 0:2, :], start=True,
                                     stop=True, perf_mode=DR)
                else:
                    nc.tensor.matmul(wps[:msz2, :], lt_sb[:, 0, 0:msz2],
                                     lt_sb[:, 0, :], start=True, stop=True)

        def ptag(gi, mo):
            # dedicated banks for the second-to-last group so its chains
            # need not wait on the previous group's consumers
            if mo2 == 2 and gi == len(groups) - 2:
                return f"q{mo % 2}"
            return f"p{mo % 4}"
        for gi, mos in enumerate(groups):
            pss = {m: ps.tile([128, rpc], F32, tag=ptag(gi, m),
                              name=f"p{m}") for m in mos}
            if use_dr:
                for kp in range(KTp // 2):
                    for mo in mos:
                        nc.tensor.matmul(
                            pss[mo][:],
                            r_sb[:, 2 * kp:2 * kp + 2,
                                 mo * 128:(mo + 1) * 128],
                            lt_sb[:, 2 * kp:2 * kp + 2, :],
                            start=(kp == 0), stop=(kp == KTp // 2 - 1),
                            perf_mode=DR)
            else:
                for kt in range(KTp):
                    for mo in mos:
                        nc.tensor.matmul(
                            pss[mo][:],
                            r_sb[:, kt, mo * 128:(mo + 1) * 128],
                            lt_sb[:, kt, :],
                            start=(kt == 0), stop=(kt == KTp - 1))
            for mo in mos:
                nc.any.tensor_copy(mt_sb[:, mo, :], pss[mo][:])
            lastg = gi == len(groups) - 1

            def gcn_mms(m, mos, stop_here):
                if mt8:
                    kpl = list(range(mos[0] // 2, (mos[-1] + 1) // 2))
                    for kp in kpl:
                        for ci, ch in enumerate((yph_sb, ypl_sb)):
                            nc.tensor.matmul(
                                gps[m][:msz2, :],
                                mt_sb[:, 2 * kp:2 * kp + 2,
                                      m * 128:m * 128 + msz2],
                                ch[:, 2 * kp:2 * kp + 2, :],
                                start=(kp == 0 and ci == 0),
                                stop=(stop_here and kp == kpl[-1]
                                      and ci == 1),
                                perf_mode=DR)
                else:
                    for kt in mos:
                        nc.tensor.matmul(
                            gps[m][:msz2, :],
                            mt_sb[:, kt, m * 128:m * 128 + msz2],
                            yp_sb[:, kt, :], start=(kt == 0),
                            stop=(stop_here and kt == mos[-1]))

            for m in range(mo2):
                gcn_mms(m, mos, lastg)
                if lastg:
                    nc.vector.tensor_scalar(
                        xo_sb[:msz2, m, :], gps[m][:msz2, :],
                        disp_sb[:msz2, m:m + 1], 0.0, MULT, MAXOP)
            if gi == 0:
                # diag/bias corrections ride mid-chain (data ready early)
                W = max(rpc, 128)
                for m in range(mo2):
                    nc.tensor.matmul(gps[m][:msz2, :],
                                     dgys_sb[:msz2, m, :msz2],
                                     dgys_sb[:msz2, m, 128:],
                                     start=False, stop=False)
                    nc.tensor.matmul(gps[m][:msz2, :],
                                     ivbv_sb[:1, m * 128:m * 128 + msz2],
                                     ivbv_sb[:1, W:], start=False,
                                     stop=False)
        if ship:
            # merge the last two (adjacent) group ships into one DMA so
            # only one issue slot sits ahead of the xo store
            spans_out = [(g[0], g[-1] + 1) for g in groups[:-2]]
            spans_out.append((groups[-2][0], groups[-1][-1] + 1))
            for o0, o1 in spans_out:
                nc.sync.dma_start(MT.ap()[:, o0:o1, :],
                                  mt_sb[:, o0:o1, :])
        if rpc >= 128:
            nc.sync.dma_start(XO.ap(), xo_sb[:])
        else:
            nc.sync.dma_start(XO.ap(), xo_sb[:rpc, 0, :])
        ctx.close()
    return _finish(nc)


# ------------------------------------------------------------------ K4b
def build_k4b():
    """xU1 = relu(N2 @ z2 + b0) replicated;
    xU2[sl] = relu((N1[sl]@xd1 + N1[sl][:,perm1]@xU1) @ Wu1 + b1).

    The middle product is computed directly in transposed form with the
    output-column dis scale deferred to the xU2 consumer:
      v2T' = (dis*xd1)^T-split-fp8 @ M1R (raw ints, diag:=2, DoubleRow)
           + xU1^T-chain @ Q1B' (dis-col-prescaled, bf16),
    so no on-device transpose is needed, the big xd1 chain runs at fp8
    DoubleRow rate, and v2T' feeds the Wu1 matmul as lhsT directly."""
    nc = _new_nc()
    NT2 = nc.dram_tensor("NT2", [128, 8, 1024], BF16, kind="ExternalInput")
    Z2 = nc.dram_tensor("Z2", [128, 8, C], BF16, kind="ExternalInput")
    XDH = nc.dram_tensor("XDH", [128, 16, C], FP8, kind="ExternalInput")
    XDL = nc.dram_tensor("XDL", [128, 16, C], FP8, kind="ExternalInput")
    NT1R = nc.dram_tensor("NT1R", [128, 16, C], FP8, kind="ExternalInput")
    Q1B = nc.dram_tensor("Q1B", [128, 8, C], BF16, kind="ExternalInput")
    WU1 = nc.dram_tensor("WU1", [128, 2, C], BF16, kind="ExternalInput")
    ONES = nc.dram_tensor("ONES", [1, 128], BF16, kind="ExternalInput")
    B0 = nc.dram_tensor("B0", [1, C], BF16, kind="ExternalInput")
    DISPQ = nc.dram_tensor("DISPQ", [128, 2], F32, kind="ExternalInput")
    IVB1 = nc.dram_tensor("IVB1", [1, C + C], BF16, kind="ExternalInput")
    XO = nc.dram_tensor("XO", [128, 2, C], F32, kind="ExternalOutput")
    with tile.TileContext(nc) as tc:
        ctx = ExitStack()
        sb = ctx.enter_context(tc.tile_pool(name="sb", bufs=1))
        ps = ctx.enter_context(tc.tile_pool(name="ps", bufs=1, space="PSUM"))
        xdh_sb = sb.tile([128, 16, C], FP8, tag="xdh")
        xdl_sb = sb.tile([128, 16, C], FP8, tag="xdl")
        nt1r_sb = sb.tile([128, 16, C], FP8, tag="nt1r")
        for k0 in range(0, 16, 8):
            _ld(nc, xdh_sb, XDH, k0, k0 + 8)
            _ld(nc, xdl_sb, XDL, k0, k0 + 8)
            _ld(nc, nt1r_sb, NT1R, k0, k0 + 8)
        z2_sb = sb.tile([128, 8, C], BF16, tag="z2")
        nc.sync.dma_start(z2_sb[:], Z2.ap())
        nt2_sb = sb.tile([128, 8, 1024], BF16, tag="nt2")
        for k0 in range(0, 8, 2):
            _ld(nc, nt2_sb, NT2, k0, k0 + 2)
        q1b_sb = sb.tile([128, 8, C], BF16, tag="q1b")
        nc.sync.dma_start(q1b_sb[:], Q1B.ap())
        ones_sb = sb.tile([128, 128], BF16, tag="ones")
        nc.sync.dma_start(ones_sb[:1, :], ONES.ap())
        b0_sb = sb.tile([128, C], BF16, tag="b0")
        nc.sync.dma_start(b0_sb[:1, :], B0.ap())
        dispq_sb = sb.tile([128, 2], F32, tag="dispq")
        nc.sync.dma_start(dispq_sb[:], DISPQ.ap())
        ivb1_sb = sb.tile([128, C + C], BF16, tag="ivb1")
        nc.sync.dma_start(ivb1_sb[:1, :], IVB1.ap())
        wu1_sb = sb.tile([128, 2, C], BF16, tag="wu1")
        nc.sync.dma_start(wu1_sb[:], WU1.ap())

        xu1_sb = sb.tile([128, 8, C], BF16, tag="xu1")
        pv = [ps.tile([128, C], F32, tag=f"v{m}", name=f"v{m}")
              for m in range(2)]
        # xd1 part: fp8 DoubleRow split against raw-int M1 (diag:=2)
        for kp in range(8):
            for ci, ch in enumerate((xdh_sb, xdl_sb)):
                for cm in range(2):
                    nc.tensor.matmul(
                        pv[cm][:],
                        ch[:, 2 * kp:2 * kp + 2,
                           cm * 128:(cm + 1) * 128],
                        nt1r_sb[:, 2 * kp:2 * kp + 2, :],
                        start=(kp == 0 and ci == 0), stop=False,
                        perf_mode=DR)
        xu1_groups = [list(range(6)), [6, 7]]
        for mos in xu1_groups:
            pss = {m: ps.tile([128, C], F32, tag=f"p{m % 6}",
                              name=f"pu{m}") for m in mos}
            for kt in range(8):
                for mo in mos:
                    nc.tensor.matmul(
                        pss[mo][:], nt2_sb[:, kt, mo * 128:(mo + 1) * 128],
                        z2_sb[:, kt, :], start=(kt == 0), stop=False)
            for mo in mos:
                nc.tensor.matmul(pss[mo][:], ones_sb[:1, :], b0_sb[:1, :],
                                 start=False, stop=True)
                nc.vector.tensor_scalar_max(xu1_sb[:, mo, :], pss[mo][:],
                                            0.0)
                for cm in range(2):
                    nc.tensor.matmul(
                        pv[cm][:], xu1_sb[:, mo, cm * 128:(cm + 1) * 128],
                        q1b_sb[:, mo, :], start=False, stop=(mo == 7))
        v2t_sb = sb.tile([128, 2, C], BF16, tag="v2t")
        for cm in range(2):
            nc.any.tensor_copy(v2t_sb[:, cm, :], pv[cm][:])
        xo_sb = sb.tile([128, 2, C], F32, tag="xo")
        for mo in range(2):
            ps3 = ps.tile([128, C], F32, tag=f"p{mo + 2}", name="pw")
            for kt in range(2):
                nc.tensor.matmul(
                    ps3[:], v2t_sb[:, kt, mo * 128:(mo + 1) * 128],
                    wu1_sb[:, kt, :], start=(kt == 0), stop=False)
            nc.tensor.matmul(ps3[:], ivb1_sb[:1, mo * 128:(mo + 1) * 128],
                             ivb1_sb[:1, C:], start=False, stop=True)
            nc.vector.tensor_scalar(xo_sb[:, mo, :], ps3[:],
                                    dispq_sb[:, mo:mo + 1], 0.0,
                                    MULT, MAXOP)
        nc.sync.dma_start(XO.ap(), xo_sb[:])
        ctx.close()
    return _finish(nc)


# =================================================================== host
F8NP = ml_dtypes.float8_e4m3fn
BFNP = ml_dtypes.bfloat16


def _pm(a, dt):
    """[K, F] row-major -> partition-major [128, K//128, F]."""
    K, F = a.shape
    return np.ascontiguousarray(
        a.reshape(K // 128, 128, F).transpose(1, 0, 2)).astype(dt)


def _unpm(b):
    """[128, KT, F] -> [K, F]."""
    p, kt, f = b.shape
    return np.asarray(b, np.float32).transpose(1, 0, 2).reshape(kt * p, f)


def _pmv(v):
    """[K] -> [128, K//128] partition-major (padded to 128 rows)."""
    k = v.shape[0]
    if k < 128:
        v = np.pad(v, (0, 128 - k))
        k = 128
    return np.ascontiguousarray(
        v.reshape(k // 128, 128).T).astype(np.float32)


def _dgys(c, ys):
    """Pack diag-correction lhsT (c on the diagonal) with its ys rows:
    [128, mo2, 128 + C] bf16."""
    rpc = c.shape[0]
    mo2 = (rpc + 127) // 128
    out = np.zeros((128, mo2, 128 + ys.shape[1]), np.float32)
    for m in range(mo2):
        seg = c[m * 128:(m + 1) * 128]
        out[np.arange(len(seg)), m, np.arange(len(seg))] = seg
        rows = ys[m * 128:(m + 1) * 128]
        out[:rows.shape[0], m, 128:] = rows
    return out.astype(BFNP)


def _ivbv(iv, b, width):
    """Pack the rank-1 bias chain row: [1, width + C] bf16."""
    out = np.zeros((1, width + b.shape[0]), np.float32)
    out[0, :iv.shape[0]] = iv
    out[0, width:] = b
    return out.astype(BFNP)


def _mk_dis(deg):
    return (1.0 / np.sqrt(np.maximum(deg, 1e-12))).astype(np.float32)


def _diag_inputs(A8T, z, dis, bvec, rpc):
    """Per-core in_maps for the K1/K4c program. A8T is [4096, 4096] fp8
    (= (A0+2I)^T); the per-core AT block ships mo-major
    [128, MO, KT, 128]."""
    zhi = z.astype(F8NP)
    zlo = (z - zhi.astype(np.float32)).astype(F8NP)
    zhi_pm, zlo_pm = _pm(zhi, F8NP), _pm(zlo, F8NP)
    n = A8T.shape[0]
    maps = []
    for c in range(NCORES):
        sl = slice(c * rpc, (c + 1) * rpc)
        blk = A8T[:, sl]                       # [n, rpc]
        at = np.ascontiguousarray(
            blk.reshape(n // 128, 128, rpc // 128, 128)
            .transpose(1, 2, 0, 3))            # [128, MO, KT, 128]
        maps.append({
            "AT": at,
            "ZH": zhi_pm, "ZL": zlo_pm,
            "DISP": _pmv(dis[sl]),
            "IVBV": _ivbv(1.0 / dis[sl], np.asarray(bvec, np.float32),
                          rpc),
            })
    return maps


def kernel(x, edge_index, W_init, b_init, W_down, b_down, p_pool,
           W_up, b_up, W_final, b_final):
    x = np.asarray(x, np.float32)
    N = x.shape[0]
    rpc0 = N // NCORES

    A0 = np.zeros((N, N), np.float32)
    np.add.at(A0, (np.asarray(edge_index[0]), np.asarray(edge_index[1])),
              1.0)
    dis0 = _mk_dis(A0.sum(1) + 2.0)
    y0 = x @ np.asarray(W_init, np.float32)

    # exact level-0 score via host matvec (init GCN is linear)
    p0 = np.asarray(p_pool[0], np.float32)
    u = y0 @ p0
    s0 = (dis0 * (A0 @ (dis0 * u)) + 2.0 * dis0 * dis0 * u) \
        / np.linalg.norm(p0)
    perm0 = np.argsort(-s0, kind="stable")[:N // 2]
    sv0 = s0[perm0]

    # ---- K1
    A2I = np.ascontiguousarray(A0.T)               # (A0+2I)^T
    A2I[np.arange(N), np.arange(N)] += 2.0
    A8T = A2I.astype(F8NP)                         # [4096, 4096]
    nc1 = build_diag()
    maps = _diag_inputs(A8T, dis0[:, None] * y0, dis0,
                        np.asarray(b_init, np.float32), rpc0)
    res = _run(nc1, maps)
    x0 = np.concatenate([_unpm(r["XO"]) for r in res], 0)

    # ---- down levels
    Bh = A0 + np.eye(N, dtype=np.float32)
    xcur, perm, sv = x0, perm0, sv0
    n = N
    Ms, dis_l, xs, perms = [], [dis0], [x0], []
    level_fp8 = [True, True, False]
    for lev in range(3):
        k = n // 2
        rpc = k // NCORES
        perms.append(perm)
        L = Bh[perm, :]
        R = Bh[:, perm]
        lim = 16 if level_fp8[lev] else 256
        assert Bh.max() <= lim, (lev, Bh.max())
        diagM = np.einsum('it,ti->i', L, R, optimize=True)
        deg = L @ R.sum(1) - diagM + 2.0
        dis = _mk_dis(deg)
        xp = xcur[perm] * np.tanh(sv)[:, None]
        y = xp @ np.asarray(W_down[lev], np.float32)
        adt = FP8 if level_fp8[lev] else BF16
        npdt = NP_OF[adt]
        ship = lev < 2
        mt8 = lev == 0
        nc = build_level(n, k, rpc, adt, ship, mt8)
        Rpm = _pm(R, npdt)
        yfull = (dis[:, None] * y).astype(np.float32)
        bvec = np.asarray(b_down[lev], np.float32)
        # compensate the low-precision rounding of the in-SBUF M diagonal
        diag_b = diagM.astype(F8NP if mt8 else BFNP).astype(np.float32)
        if mt8:
            yph = yfull.astype(F8NP)
            ypl = (yfull - yph.astype(np.float32)).astype(F8NP)
            ypmaps = {"YPH": _pm(yph, F8NP), "YPL": _pm(ypl, F8NP)}
        else:
            ypmaps = {"YP": _pm(yfull, BFNP)}
        maps = []
        for cc in range(NCORES):
            sl = slice(cc * rpc, (cc + 1) * rpc)
            maps.append({
                "R": Rpm,
                "LT": _pm(np.ascontiguousarray(L[sl].T), npdt),
                **ypmaps,
                "DISP": _pmv(dis[sl]),
                "DGYS": _dgys(2.0 - diag_b[sl], yfull[sl]),
                "IVBV": _ivbv(1.0 / dis[sl], bvec, max(rpc, 128)),
                })
        res = _run(nc, maps)
        if rpc >= 128:
            xn = np.concatenate([_unpm(r["XO"]) for r in res], 0)
        else:
            xn = np.concatenate([np.asarray(r["XO"], np.float32)
                                 for r in res], 0)
        if ship:
            M = np.concatenate([_unpm(r["MT"]).T for r in res], 0)
            Ms.append(M)
            Bh = M.copy()
            np.fill_diagonal(Bh, 1.0)
        dis_l.append(dis)
        xs.append(xn)
        xcur, n = xn, k
        if lev < 2:
            pl = np.asarray(p_pool[lev + 1], np.float32)
            s = xn @ pl / np.linalg.norm(pl)
            perm = np.argsort(-s, kind="stable")[:k // 2]
            sv = s[perm]

    x_d1, x_d2, x_d3 = xs[1], xs[2], xs[3]
    dis1, dis2 = dis_l[1], dis_l[2]
    M1, M2 = Ms[0], Ms[1]
    perm1, perm2 = perms[1], perms[2]

    # ---- K4b
    N2 = M2.copy()
    np.fill_diagonal(N2, 2.0)
    N2 *= dis2[:, None] * dis2[None, :]
    N1 = M1.copy()
    np.fill_diagonal(N1, 2.0)
    N1 *= dis1[:, None] * dis1[None, :]
    up = np.zeros_like(x_d2)
    up[perm2] = x_d3
    z2 = (x_d2 + up) @ np.asarray(W_up[0], np.float32)
    nc4b = build_k4b()
    rpc1 = 2048 // NCORES
    nt2_pm = _pm(np.ascontiguousarray(N2.T), BFNP)
    z2_pm = _pm(z2, BFNP)
    xds = dis1[:, None] * x_d1
    xdh = xds.astype(F8NP)
    xdl = (xds - xdh.astype(np.float32)).astype(F8NP)
    xdh_pm, xdl_pm = _pm(xdh, F8NP), _pm(xdl, F8NP)
    wu1_pm = _pm(np.asarray(W_up[1], np.float32), BFNP)
    ones = np.ones((1, 128), BFNP)
    b0 = np.asarray(b_up[0], np.float32)[None, :].astype(BFNP)
    b1v = np.asarray(b_up[1], np.float32)
    maps = []
    for cc in range(NCORES):
        sl = slice(cc * rpc1, (cc + 1) * rpc1)
        nt1r = np.ascontiguousarray(M1[sl].T)      # raw ints, diag := 2
        nt1r[cc * rpc1 + np.arange(rpc1), np.arange(rpc1)] = 2.0
        maps.append({
            "NT2": nt2_pm, "Z2": z2_pm,
            "XDH": xdh_pm, "XDL": xdl_pm,
            "NT1R": _pm(nt1r, F8NP),
            "Q1B": _pm(np.ascontiguousarray(
                (N1[sl][:, perm1] / dis1[sl][:, None]).T), BFNP),
            "WU1": wu1_pm, "ONES": ones, "B0": b0,
            "DISPQ": _pmv(dis1[sl]),
            "IVB1": _ivbv(1.0 / dis1[sl], b1v, C),
            })
    res = _run(nc4b, maps)
    xU2 = np.concatenate([_unpm(r["XO"]) for r in res], 0)

    # ---- K4c
    upf = np.zeros_like(x0)
    upf[perm0] = xU2
    zf = (x0 + upf) @ np.asarray(W_final, np.float32)
    nc4c = build_diag()
    maps = _diag_inputs(A8T, dis0[:, None] * zf, dis0,
                        np.asarray(b_final, np.float32), rpc0)
    res = _run(nc4c, maps)
    out = np.concatenate([_unpm(r["XO"]) for r in res], 0)
    return out.astype(np.float32)


# revision 63
# speedup vs baseline: 1.0025x; 1.0004x over previous
"""GraphUNet (N=4096, E=65536, C=256, depth 3, ratio 0.5) on 8 trn2 NeuronCores.

Row-sharded SPMD pipeline, 6 launches. Device does the O(n^2 C) message
passing (N @ Z SpMMs) and the dense A@A augmentations; host does top-k,
gathers, O(nC^2) feature transforms / weight folds, and O(n^2)
element-wise adjacency prep between launches.

Layout/overlap notes: every DRAM tensor ships partition-major
([128, K/128, F]) so DMA descriptors move >=512B contiguous runs at the
full modeled 360 GB/s; input DMAs are emitted in consumption order with
the streamed operand chunk-interleaved so the PE starts ~4us in; output
DMAs are emitted last (the DMA queue is in-order, so a store's semaphore
wait would block later loads); diag/bias GCN corrections are folded into
the PSUM accumulation as rank-1 / diagonal matmul chains, leaving one
fused tensor_scalar per output block.

  K1   init GCN:   x0[sl] = dis0*(A0[sl] @ (zhi+zlo) + 2*dis0*y0[sl]
                   + (1/dis0) x b), A0 in exact small-int fp8 (DoubleRow),
                   z = dis0*(x@Wi) as an fp8 hi+lo split.
  K2-K4a levels:   M^T col-block = R^T @ L[sl]^T chain in fp8 (exact ints;
                   bf16 at level 3), shipped back raw (ints, fp8/bf16
                   exact off-diag; diag rounding host-compensated in the
                   in-chain diag correction); the down GCN reuses the
                   in-SBUF M^T block as lhsT against host-prescaled
                   yp = dis*(xp@Wd) (fp8 hi+lo DoubleRow at level 1),
                   with diag/bias corrections in-chain and relu fused
                   into the dis row-scale consumer.
  K4b  up GCNs:    xU1 = relu(N2 @ z2 + b) replicated (z2 host-folds the
                   deepest unpool scatter); the middle product is built
                   directly in transposed form (lhsT/rhs roles swapped)
                   so no on-device transpose is needed, with its xd1 part
                   running fp8 DoubleRow against raw-int M1 (diag := 2)
                   and the output-column dis scale deferred to the xU2
                   consumer; biases enter psums via rank-1 ones chains.
  K4c  final GCN:  identical program to K1 on zf = (x0 + scatter(perm0,
                   xU2)) @ Wf -- the unpool scatter and Wf fold on host
                   collapse the two chains of the reference into one.

Precision: adjacency chains exact; features bf16 (fp8 hi+lo where they
feed the big A-chains); measured end-to-end rel err ~1.1e-2 vs f32 ref.
"""

import numpy as np
import ml_dtypes

from contextlib import ExitStack

import concourse.bass as bass
import concourse.mybir as mybir
import concourse.tile as tile
from concourse import bacc
from concourse.bass_utils import run_bass_kernel_spmd

NCORES = 8
C = 256
F32 = mybir.dt.float32
F32R = mybir.dt.float32r
BF16 = mybir.dt.bfloat16
FP8 = mybir.dt.float8e4

NP_OF = {F32: np.float32, F32R: np.float32,
         BF16: ml_dtypes.bfloat16, FP8: ml_dtypes.float8_e4m3fn}

_TRACE = {"on": False, "results": [], "ncs": []}
DR = mybir.MatmulPerfMode.DoubleRow
MULT = mybir.AluOpType.mult
MAXOP = mybir.AluOpType.max


def _new_nc():
    return bacc.Bacc("TRN2", target_bir_lowering=False, debug=False,
                     num_devices=NCORES)


def _finish(nc):
    nc.compile()
    _TRACE["ncs"].append(nc)
    return nc


def _run(nc, in_maps):
    res = run_bass_kernel_spmd(nc, in_maps, list(range(NCORES)),
                               trace=_TRACE["on"])
    if _TRACE["on"]:
        _TRACE["results"].append(res)
    return res.results


def _ld(nc, t, dram, k0, k1, f0=None, f1=None):
    if f0 is None:
        nc.sync.dma_start(t[:, k0:k1, :], dram.ap()[:, k0:k1, :])
    else:
        nc.sync.dma_start(t[:, k0:k1, f0:f1], dram.ap()[:, k0:k1, f0:f1])


# ------------------------------------------------------------ K1 / K4c
def build_diag():
    """x[sl] = dis[sl]*((A0+2I)[sl] @ (zhi+zlo)) + (1/dis)xb scaled.
    [4096 -> 512/core].  The +2I diagonal folds into the fp8 adjacency
    (still exact small ints).

    AT ships mo-major so each 128-row output chain completes as soon as
    its A-slice lands; consumers and stores pipeline behind the PE."""
    n, rpc, KT, MO = 4096, 512, 32, 4
    nc = _new_nc()
    AT = nc.dram_tensor("AT", [128, MO, KT, 128], FP8,
                        kind="ExternalInput")
    ZH = nc.dram_tensor("ZH", [128, KT, C], FP8, kind="ExternalInput")
    ZL = nc.dram_tensor("ZL", [128, KT, C], FP8, kind="ExternalInput")
    DISP = nc.dram_tensor("DISP", [128, MO], F32, kind="ExternalInput")
    IVBV = nc.dram_tensor("IVBV", [1, rpc + C], BF16,
                          kind="ExternalInput")
    XO = nc.dram_tensor("XO", [128, MO, C], F32, kind="ExternalOutput")
    with tile.TileContext(nc) as tc:
        ctx = ExitStack()
        sb = ctx.enter_context(tc.tile_pool(name="sb", bufs=1))
        ps = ctx.enter_context(tc.tile_pool(name="ps", bufs=1, space="PSUM"))
        at_sb = sb.tile([128, MO, KT, 128], FP8, tag="at")
        zh_sb = sb.tile([128, KT, C], FP8, tag="zh")
        zl_sb = sb.tile([128, KT, C], FP8, tag="zl")
        disp_sb = sb.tile([128, MO], F32, tag="disp")
        ivbv_sb = sb.tile([128, rpc + C], BF16, tag="ivbv")
        nc.sync.dma_start(at_sb[:, 0, :, :], AT.ap()[:, 0, :, :])
        nc.sync.dma_start(disp_sb[:], DISP.ap())
        nc.sync.dma_start(ivbv_sb[:1, :], IVBV.ap())
        _ld(nc, zh_sb, ZH, 0, 8)
        _ld(nc, zl_sb, ZL, 0, 8)
        nc.sync.dma_start(at_sb[:, 1, :, :], AT.ap()[:, 1, :, :])
        _ld(nc, zh_sb, ZH, 8, 16)
        _ld(nc, zl_sb, ZL, 8, 16)
        nc.sync.dma_start(at_sb[:, 2, :, :], AT.ap()[:, 2, :, :])
        _ld(nc, zh_sb, ZH, 16, 24)
        _ld(nc, zl_sb, ZL, 16, 24)
        _ld(nc, zh_sb, ZH, 24, 32)
        _ld(nc, zl_sb, ZL, 24, 32)
        nc.sync.dma_start(at_sb[:, 3, :, :], AT.ap()[:, 3, :, :])
        xo_sb = sb.tile([128, MO, C], F32, tag="xo")
        pss = [ps.tile([128, C], F32, tag=f"p{m}", name=f"p{m}")
               for m in range(MO)]
        # p-state warmers: keep the PE continuously busy through the load
        # stream so the real chains run at the fully-ramped clock
        wps = ps.tile([128, 128], F32, tag="w", name="w")
        for _ in range(60):
            nc.tensor.matmul(wps[:], at_sb[:, 0, 0:2, :],
                             at_sb[:, 0, 0:2, :], start=True, stop=True,
                             perf_mode=DR)

        def chain(mo, kps, first, last):
            for kp in kps:
                for ci, ch in enumerate((zh_sb, zl_sb)):
                    nc.tensor.matmul(
                        pss[mo][:], at_sb[:, mo, 2 * kp:2 * kp + 2, :],
                        ch[:, 2 * kp:2 * kp + 2, :],
                        start=(first and kp == kps[0] and ci == 0),
                        stop=False, perf_mode=DR)
            if last:
                nc.tensor.matmul(pss[mo][:],
                                 ivbv_sb[:1, mo * 128:(mo + 1) * 128],
                                 ivbv_sb[:1, rpc:], start=False, stop=True)
                nc.any.tensor_scalar_mul(xo_sb[:, mo, :], pss[mo][:],
                                         disp_sb[:, mo:mo + 1])

        for mo in range(MO):
            chain(mo, list(range(16)), True, True)
        for mo in range(MO):
            nc.sync.dma_start(XO.ap()[:, mo, :], xo_sb[:, mo, :])
        ctx.close()
    return _finish(nc)


# ----------------------------------------------------- K2 / K3 / K4a
def build_level(npv, n, rpc, adt, ship, mt8=False):
    """M^T col-block = R^T @ L[sl]^T (exact ints), then
    x[sl] = relu(dis[sl] * (M[sl] @ yp + diag(2-diagM) yp[sl]
    + (1/dis) x b)).  Ships raw M^T if `ship`.  With mt8, the M block
    stays fp8 (offdiag ints <= 16 exact, diag host-compensated) and the
    GCN runs fp8 DoubleRow over an yp hi+lo split."""
    KTp, KT = npv // 128, n // 128
    mo2 = (rpc + 127) // 128
    mdt = FP8 if mt8 else BF16
    nc = _new_nc()
    R = nc.dram_tensor("R", [128, KTp, n], adt, kind="ExternalInput")
    LT = nc.dram_tensor("LT", [128, KTp, rpc], adt, kind="ExternalInput")
    if mt8:
        YPH = nc.dram_tensor("YPH", [128, KT, C], FP8,
                             kind="ExternalInput")
        YPL = nc.dram_tensor("YPL", [128, KT, C], FP8,
                             kind="ExternalInput")
    else:
        YP = nc.dram_tensor("YP", [128, KT, C], BF16,
                            kind="ExternalInput")
    DISP = nc.dram_tensor("DISP", [128, mo2], F32, kind="ExternalInput")
    # DGYS packs the diag-correction lhsT (128) and its ys rows (C)
    DGYS = nc.dram_tensor("DGYS", [128, mo2, 128 + C], BF16,
                          kind="ExternalInput")
    # IVBV packs the rank-1 bias chain: row0 = [1/dis[sl] | b]
    IVBV = nc.dram_tensor("IVBV", [1, max(rpc, 128) + C], BF16,
                          kind="ExternalInput")
    if ship:
        MT = nc.dram_tensor("MT", [128, KT, rpc], mdt,
                            kind="ExternalOutput")
    XO = nc.dram_tensor("XO", [128, mo2, C] if rpc >= 128 else [rpc, C],
                        F32, kind="ExternalOutput")
    # PSUM groups of <=4 chain banks (+ mo2 GCN banks), tail split into
    # 2-block groups so the last chain+copy+ship is short. R loads use
    # separate f-spans (unions of groups) so each DMA moves >=512B
    # contiguous runs.
    groups = []
    rem = list(range(KT))
    while rem:
        take = 4 if len(rem) > 4 else 2
        groups.append(rem[:take])
        rem = rem[take:]
    span_blocks = max(1, 512 // (128 * mybir.dt.size(adt)))
    load_spans = []
    cur = None
    for g in groups:
        cur = [g[0], g[-1] + 1] if cur is None else [cur[0], g[-1] + 1]
        if cur[1] - cur[0] >= span_blocks:
            load_spans.append(tuple(cur))
            cur = None
    if cur is not None:
        if load_spans:
            load_spans[-1] = (load_spans[-1][0], cur[1])
        else:
            load_spans.append(tuple(cur))
    with tile.TileContext(nc) as tc:
        ctx = ExitStack()
        sb = ctx.enter_context(tc.tile_pool(name="sb", bufs=1))
        ps = ctx.enter_context(tc.tile_pool(name="ps", bufs=1, space="PSUM"))
        disp_sb = sb.tile([128, mo2], F32, tag="disp")
        dgys_sb = sb.tile([128, mo2, 128 + C], BF16, tag="dgys")
        ivbv_sb = sb.tile([128, max(rpc, 128) + C], BF16, tag="ivbv")
        lt_sb = sb.tile([128, KTp, rpc], adt, tag="lt")
        r_sb = sb.tile([128, KTp, n], adt, tag="r")
        if mt8:
            yph_sb = sb.tile([128, KT, C], FP8, tag="yph")
            ypl_sb = sb.tile([128, KT, C], FP8, tag="ypl")
        else:
            yp_sb = sb.tile([128, KT, C], BF16, tag="yp")
        for si, (b0, b1) in enumerate(load_spans):
            f0, f1 = b0 * 128, b1 * 128
            if si == len(load_spans) - 1:
                # finer chunks at the stream tail: less work gated on
                # the final bytes
                kchunks = [(k, min(KTp, k + 8)) for k in
                           range(0, max(0, KTp - 8), 8)] + \
                    [(max(0, KTp - 8), max(0, KTp - 4)), (KTp - 4, KTp)]
            else:
                kchunks = [(k, min(KTp, k + 8)) for k in range(0, KTp, 8)]
            for k0, k1 in kchunks:
                if si == 0:
                    _ld(nc, lt_sb, LT, k0, k1)
                _ld(nc, r_sb, R, k0, k1, f0, f1)
            if si == 0:
                if mt8:
                    for k0 in range(0, KT, 8):
                        _ld(nc, yph_sb, YPH, k0, min(KT, k0 + 8))
                        _ld(nc, ypl_sb, YPL, k0, min(KT, k0 + 8))
                else:
                    for k0 in range(0, KT, 8):
                        _ld(nc, yp_sb, YP, k0, min(KT, k0 + 8))
        nc.sync.dma_start(disp_sb[:], DISP.ap())
        nc.sync.dma_start(dgys_sb[:], DGYS.ap())
        nc.sync.dma_start(ivbv_sb[:1, :], IVBV.ap())
        mt_sb = sb.tile([128, KT, rpc], mdt, tag="mt")
        xo_sb = sb.tile([128, mo2, C], F32, tag="xo")
        use_dr = adt == FP8
        gps = [ps.tile([128, C], F32, tag=f"g{m}", name=f"g{m}")
               for m in range(mo2)]
        msz2 = min(128, rpc)
        # p-state warmers (lowest-priority filler on LT data): keep the PE
        # ramped to full clock through the arrival-gated M-chain bursts.
        # With mo2 == 2 the banks go to dedicated second-to-last-group
        # psums instead (warmers measured no effect there).
        if mo2 < 2:
            wps = ps.tile([128, rpc], F32, tag="w", name="w")
            for _ in range(40 if KTp <= 16 else 80):
                if use_dr:
                    nc.tensor.matmul(wps[:msz2, :], lt_sb[:, 0:2, 0:msz2],
                                     lt_sb[:, 0:2, :], start=True,
                                     stop=True, perf_mode=DR)
                else:
                    nc.tensor.matmul(wps[:msz2, :], lt_sb[:, 0, 0:msz2],
                                     lt_sb[:, 0, :], start=True, stop=True)

        def ptag(gi, mo):
            # dedicated banks for the second-to-last group so its chains
            # need not wait on the previous group's consumers
            if mo2 == 2 and gi == len(groups) - 2:
                return f"q{mo % 2}"
            return f"p{mo % 4}"
        for gi, mos in enumerate(groups):
            pss = {m: ps.tile([128, rpc], F32, tag=ptag(gi, m),
                              name=f"p{m}") for m in mos}
            if use_dr:
                for kp in range(KTp // 2):
                    for mo in mos:
                        nc.tensor.matmul(
                            pss[mo][:],
                            r_sb[:, 2 * kp:2 * kp + 2,
                                 mo * 128:(mo + 1) * 128],
                            lt_sb[:, 2 * kp:2 * kp + 2, :],
                            start=(kp == 0), stop=(kp == KTp // 2 - 1),
                            perf_mode=DR)
            else:
                for kt in range(KTp):
                    for mo in mos:
                        nc.tensor.matmul(
                            pss[mo][:],
                            r_sb[:, kt, mo * 128:(mo + 1) * 128],
                            lt_sb[:, kt, :],
                            start=(kt == 0), stop=(kt == KTp - 1))
            for mo in mos:
                nc.any.tensor_copy(mt_sb[:, mo, :], pss[mo][:])
            lastg = gi == len(groups) - 1

            def gcn_mms(m, mos, stop_here):
                if mt8:
                    kpl = list(range(mos[0] // 2, (mos[-1] + 1) // 2))
                    for kp in kpl:
                        for ci, ch in enumerate((yph_sb, ypl_sb)):
                            nc.tensor.matmul(
                                gps[m][:msz2, :],
                                mt_sb[:, 2 * kp:2 * kp + 2,
                                      m * 128:m * 128 + msz2],
                                ch[:, 2 * kp:2 * kp + 2, :],
                                start=(kp == 0 and ci == 0),
                                stop=(stop_here and kp == kpl[-1]
                                      and ci == 1),
                                perf_mode=DR)
                else:
                    for kt in mos:
                        nc.tensor.matmul(
                            gps[m][:msz2, :],
                            mt_sb[:, kt, m * 128:m * 128 + msz2],
                            yp_sb[:, kt, :], start=(kt == 0),
                            stop=(stop_here and kt == mos[-1]))

            for m in range(mo2):
                gcn_mms(m, mos, lastg)
                if lastg:
                    nc.vector.tensor_scalar(
                        xo_sb[:msz2, m, :], gps[m][:msz2, :],
                        disp_sb[:msz2, m:m + 1], 0.0, MULT, MAXOP)
            if gi == 0:
                # diag/bias corrections ride mid-chain (data ready early)
                W = max(rpc, 128)
                for m in range(mo2):
                    nc.tensor.matmul(gps[m][:msz2, :],
                                     dgys_sb[:msz2, m, :msz2],
                                     dgys_sb[:msz2, m, 128:],
                                     start=False, stop=False)
                    nc.tensor.matmul(gps[m][:msz2, :],
                                     ivbv_sb[:1, m * 128:m * 128 + msz2],
                                     ivbv_sb[:1, W:], start=False,
                                     stop=False)
        if ship:
            # two ships only (split at the group edge nearest KT/2): just
            # one issue slot sits ahead of the xo store in the in-order
            # store queue
            edges = [g[-1] + 1 for g in groups[:-1]]
            mid = min(edges, key=lambda e: abs(e - KT // 2))
            for o0, o1 in ((0, mid), (mid, KT)):
                nc.sync.dma_start(MT.ap()[:, o0:o1, :],
                                  mt_sb[:, o0:o1, :])
        if rpc >= 128:
            nc.sync.dma_start(XO.ap(), xo_sb[:])
        else:
            nc.sync.dma_start(XO.ap(), xo_sb[:rpc, 0, :])
        ctx.close()
    return _finish(nc)


# ------------------------------------------------------------------ K4b
def build_k4b():
    """xU1 = relu(N2 @ z2 + b0) replicated;
    xU2[sl] = relu((N1[sl]@xd1 + N1[sl][:,perm1]@xU1) @ Wu1 + b1).

    The middle product is computed directly in transposed form with the
    output-column dis scale deferred to the xU2 consumer:
      v2T' = (dis*xd1)^T-split-fp8 @ M1R (raw ints, diag:=2, DoubleRow)
           + xU1^T-chain @ Q1B' (dis-col-prescaled, bf16),
    so no on-device transpose is needed, the big xd1 chain runs at fp8
    DoubleRow rate, and v2T' feeds the Wu1 matmul as lhsT directly."""
    nc = _new_nc()
    NT2 = nc.dram_tensor("NT2", [128, 8, 1024], BF16, kind="ExternalInput")
    Z2 = nc.dram_tensor("Z2", [128, 8, C], BF16, kind="ExternalInput")
    XDH = nc.dram_tensor("XDH", [128, 16, C], FP8, kind="ExternalInput")
    XDL = nc.dram_tensor("XDL", [128, 16, C], FP8, kind="ExternalInput")
    NT1R = nc.dram_tensor("NT1R", [128, 16, C], FP8, kind="ExternalInput")
    Q1B = nc.dram_tensor("Q1B", [128, 8, C], BF16, kind="ExternalInput")
    WU1 = nc.dram_tensor("WU1", [128, 2, C], BF16, kind="ExternalInput")
    ONES = nc.dram_tensor("ONES", [1, 128], BF16, kind="ExternalInput")
    B0 = nc.dram_tensor("B0", [1, C], BF16, kind="ExternalInput")
    DISPQ = nc.dram_tensor("DISPQ", [128, 2], F32, kind="ExternalInput")
    IVB1 = nc.dram_tensor("IVB1", [1, C + C], BF16, kind="ExternalInput")
    XO = nc.dram_tensor("XO", [128, 2, C], F32, kind="ExternalOutput")
    with tile.TileContext(nc) as tc:
        ctx = ExitStack()
        sb = ctx.enter_context(tc.tile_pool(name="sb", bufs=1))
        ps = ctx.enter_context(tc.tile_pool(name="ps", bufs=1, space="PSUM"))
        xdh_sb = sb.tile([128, 16, C], FP8, tag="xdh")
        xdl_sb = sb.tile([128, 16, C], FP8, tag="xdl")
        nt1r_sb = sb.tile([128, 16, C], FP8, tag="nt1r")
        for k0 in range(0, 16, 8):
            _ld(nc, xdh_sb, XDH, k0, k0 + 8)
            _ld(nc, xdl_sb, XDL, k0, k0 + 8)
            _ld(nc, nt1r_sb, NT1R, k0, k0 + 8)
        z2_sb = sb.tile([128, 8, C], BF16, tag="z2")
        nc.sync.dma_start(z2_sb[:], Z2.ap())
        nt2_sb = sb.tile([128, 8, 1024], BF16, tag="nt2")
        for k0 in range(0, 8, 2):
            _ld(nc, nt2_sb, NT2, k0, k0 + 2)
        q1b_sb = sb.tile([128, 8, C], BF16, tag="q1b")
        nc.sync.dma_start(q1b_sb[:], Q1B.ap())
        ones_sb = sb.tile([128, 128], BF16, tag="ones")
        nc.sync.dma_start(ones_sb[:1, :], ONES.ap())
        b0_sb = sb.tile([128, C], BF16, tag="b0")
        nc.sync.dma_start(b0_sb[:1, :], B0.ap())
        dispq_sb = sb.tile([128, 2], F32, tag="dispq")
        nc.sync.dma_start(dispq_sb[:], DISPQ.ap())
        ivb1_sb = sb.tile([128, C + C], BF16, tag="ivb1")
        nc.sync.dma_start(ivb1_sb[:1, :], IVB1.ap())
        wu1_sb = sb.tile([128, 2, C], BF16, tag="wu1")
        nc.sync.dma_start(wu1_sb[:], WU1.ap())

        xu1_sb = sb.tile([128, 8, C], BF16, tag="xu1")
        pv = [ps.tile([128, C], F32, tag=f"v{m}", name=f"v{m}")
              for m in range(2)]
        # xd1 part: fp8 DoubleRow split against raw-int M1 (diag:=2)
        for kp in range(8):
            for ci, ch in enumerate((xdh_sb, xdl_sb)):
                for cm in range(2):
                    nc.tensor.matmul(
                        pv[cm][:],
                        ch[:, 2 * kp:2 * kp + 2,
                           cm * 128:(cm + 1) * 128],
                        nt1r_sb[:, 2 * kp:2 * kp + 2, :],
                        start=(kp == 0 and ci == 0), stop=False,
                        perf_mode=DR)
        xu1_groups = [list(range(6)), [6, 7]]
        for mos in xu1_groups:
            pss = {m: ps.tile([128, C], F32, tag=f"p{m % 6}",
                              name=f"pu{m}") for m in mos}
            for kt in range(8):
                for mo in mos:
                    nc.tensor.matmul(
                        pss[mo][:], nt2_sb[:, kt, mo * 128:(mo + 1) * 128],
                        z2_sb[:, kt, :], start=(kt == 0), stop=False)
            for mo in mos:
                nc.tensor.matmul(pss[mo][:], ones_sb[:1, :], b0_sb[:1, :],
                                 start=False, stop=True)
                nc.vector.tensor_scalar_max(xu1_sb[:, mo, :], pss[mo][:],
                                            0.0)
                for cm in range(2):
                    nc.tensor.matmul(
                        pv[cm][:], xu1_sb[:, mo, cm * 128:(cm + 1) * 128],
                        q1b_sb[:, mo, :], start=False, stop=(mo == 7))
        v2t_sb = sb.tile([128, 2, C], BF16, tag="v2t")
        for cm in range(2):
            nc.any.tensor_copy(v2t_sb[:, cm, :], pv[cm][:])
        xo_sb = sb.tile([128, 2, C], F32, tag="xo")
        for mo in range(2):
            ps3 = ps.tile([128, C], F32, tag=f"p{mo + 2}", name="pw")
            for kt in range(2):
                nc.tensor.matmul(
                    ps3[:], v2t_sb[:, kt, mo * 128:(mo + 1) * 128],
                    wu1_sb[:, kt, :], start=(kt == 0), stop=False)
            nc.tensor.matmul(ps3[:], ivb1_sb[:1, mo * 128:(mo + 1) * 128],
                             ivb1_sb[:1, C:], start=False, stop=True)
            nc.vector.tensor_scalar(xo_sb[:, mo, :], ps3[:],
                                    dispq_sb[:, mo:mo + 1], 0.0,
                                    MULT, MAXOP)
        nc.sync.dma_start(XO.ap(), xo_sb[:])
        ctx.close()
    return _finish(nc)


# =================================================================== host
F8NP = ml_dtypes.float8_e4m3fn
BFNP = ml_dtypes.bfloat16


def _pm(a, dt):
    """[K, F] row-major -> partition-major [128, K//128, F]."""
    K, F = a.shape
    return np.ascontiguousarray(
        a.reshape(K // 128, 128, F).transpose(1, 0, 2)).astype(dt)


def _unpm(b):
    """[128, KT, F] -> [K, F]."""
    p, kt, f = b.shape
    return np.asarray(b, np.float32).transpose(1, 0, 2).reshape(kt * p, f)


def _pmv(v):
    """[K] -> [128, K//128] partition-major (padded to 128 rows)."""
    k = v.shape[0]
    if k < 128:
        v = np.pad(v, (0, 128 - k))
        k = 128
    return np.ascontiguousarray(
        v.reshape(k // 128, 128).T).astype(np.float32)


def _dgys(c, ys):
    """Pack diag-correction lhsT (c on the diagonal) with its ys rows:
    [128, mo2, 128 + C] bf16."""
    rpc = c.shape[0]
    mo2 = (rpc + 127) // 128
    out = np.zeros((128, mo2, 128 + ys.shape[1]), np.float32)
    for m in range(mo2):
        seg = c[m * 128:(m + 1) * 128]
        out[np.arange(len(seg)), m, np.arange(len(seg))] = seg
        rows = ys[m * 128:(m + 1) * 128]
        out[:rows.shape[0], m, 128:] = rows
    return out.astype(BFNP)


def _ivbv(iv, b, width):
    """Pack the rank-1 bias chain row: [1, width + C] bf16."""
    out = np.zeros((1, width + b.shape[0]), np.float32)
    out[0, :iv.shape[0]] = iv
    out[0, width:] = b
    return out.astype(BFNP)


def _mk_dis(deg):
    return (1.0 / np.sqrt(np.maximum(deg, 1e-12))).astype(np.float32)


def _diag_inputs(A8T, z, dis, bvec, rpc):
    """Per-core in_maps for the K1/K4c program. A8T is [4096, 4096] fp8
    (= (A0+2I)^T); the per-core AT block ships mo-major
    [128, MO, KT, 128]."""
    zhi = z.astype(F8NP)
    zlo = (z - zhi.astype(np.float32)).astype(F8NP)
    zhi_pm, zlo_pm = _pm(zhi, F8NP), _pm(zlo, F8NP)
    n = A8T.shape[0]
    maps = []
    for c in range(NCORES):
        sl = slice(c * rpc, (c + 1) * rpc)
        blk = A8T[:, sl]                       # [n, rpc]
        at = np.ascontiguousarray(
            blk.reshape(n // 128, 128, rpc // 128, 128)
            .transpose(1, 2, 0, 3))            # [128, MO, KT, 128]
        maps.append({
            "AT": at,
            "ZH": zhi_pm, "ZL": zlo_pm,
            "DISP": _pmv(dis[sl]),
            "IVBV": _ivbv(1.0 / dis[sl], np.asarray(bvec, np.float32),
                          rpc),
            })
    return maps


def kernel(x, edge_index, W_init, b_init, W_down, b_down, p_pool,
           W_up, b_up, W_final, b_final):
    x = np.asarray(x, np.float32)
    N = x.shape[0]
    rpc0 = N // NCORES

    A0 = np.zeros((N, N), np.float32)
    np.add.at(A0, (np.asarray(edge_index[0]), np.asarray(edge_index[1])),
              1.0)
    dis0 = _mk_dis(A0.sum(1) + 2.0)
    y0 = x @ np.asarray(W_init, np.float32)

    # exact level-0 score via host matvec (init GCN is linear)
    p0 = np.asarray(p_pool[0], np.float32)
    u = y0 @ p0
    s0 = (dis0 * (A0 @ (dis0 * u)) + 2.0 * dis0 * dis0 * u) \
        / np.linalg.norm(p0)
    perm0 = np.argsort(-s0, kind="stable")[:N // 2]
    sv0 = s0[perm0]

    # ---- K1
    A2I = np.ascontiguousarray(A0.T)               # (A0+2I)^T
    A2I[np.arange(N), np.arange(N)] += 2.0
    A8T = A2I.astype(F8NP)                         # [4096, 4096]
    nc1 = build_diag()
    maps = _diag_inputs(A8T, dis0[:, None] * y0, dis0,
                        np.asarray(b_init, np.float32), rpc0)
    res = _run(nc1, maps)
    x0 = np.concatenate([_unpm(r["XO"]) for r in res], 0)

    # ---- down levels
    Bh = A0 + np.eye(N, dtype=np.float32)
    xcur, perm, sv = x0, perm0, sv0
    n = N
    Ms, dis_l, xs, perms = [], [dis0], [x0], []
    level_fp8 = [True, True, False]
    for lev in range(3):
        k = n // 2
        rpc = k // NCORES
        perms.append(perm)
        L = Bh[perm, :]
        R = Bh[:, perm]
        lim = 16 if level_fp8[lev] else 256
        assert Bh.max() <= lim, (lev, Bh.max())
        diagM = np.einsum('it,ti->i', L, R, optimize=True)
        deg = L @ R.sum(1) - diagM + 2.0
        dis = _mk_dis(deg)
        xp = xcur[perm] * np.tanh(sv)[:, None]
        y = xp @ np.asarray(W_down[lev], np.float32)
        adt = FP8 if level_fp8[lev] else BF16
        npdt = NP_OF[adt]
        ship = lev < 2
        mt8 = lev == 0
        nc = build_level(n, k, rpc, adt, ship, mt8)
        Rpm = _pm(R, npdt)
        yfull = (dis[:, None] * y).astype(np.float32)
        bvec = np.asarray(b_down[lev], np.float32)
        # compensate the low-precision rounding of the in-SBUF M diagonal
        diag_b = diagM.astype(F8NP if mt8 else BFNP).astype(np.float32)
        if mt8:
            yph = yfull.astype(F8NP)
            ypl = (yfull - yph.astype(np.float32)).astype(F8NP)
            ypmaps = {"YPH": _pm(yph, F8NP), "YPL": _pm(ypl, F8NP)}
        else:
            ypmaps = {"YP": _pm(yfull, BFNP)}
        maps = []
        for cc in range(NCORES):
            sl = slice(cc * rpc, (cc + 1) * rpc)
            maps.append({
                "R": Rpm,
                "LT": _pm(np.ascontiguousarray(L[sl].T), npdt),
                **ypmaps,
                "DISP": _pmv(dis[sl]),
                "DGYS": _dgys(2.0 - diag_b[sl], yfull[sl]),
                "IVBV": _ivbv(1.0 / dis[sl], bvec, max(rpc, 128)),
                })
        res = _run(nc, maps)
        if rpc >= 128:
            xn = np.concatenate([_unpm(r["XO"]) for r in res], 0)
        else:
            xn = np.concatenate([np.asarray(r["XO"], np.float32)
                                 for r in res], 0)
        if ship:
            M = np.concatenate([_unpm(r["MT"]).T for r in res], 0)
            Ms.append(M)
            Bh = M.copy()
            np.fill_diagonal(Bh, 1.0)
        dis_l.append(dis)
        xs.append(xn)
        xcur, n = xn, k
        if lev < 2:
            pl = np.asarray(p_pool[lev + 1], np.float32)
            s = xn @ pl / np.linalg.norm(pl)
            perm = np.argsort(-s, kind="stable")[:k // 2]
            sv = s[perm]

    x_d1, x_d2, x_d3 = xs[1], xs[2], xs[3]
    dis1, dis2 = dis_l[1], dis_l[2]
    M1, M2 = Ms[0], Ms[1]
    perm1, perm2 = perms[1], perms[2]

    # ---- K4b
    N2 = M2.copy()
    np.fill_diagonal(N2, 2.0)
    N2 *= dis2[:, None] * dis2[None, :]
    N1 = M1.copy()
    np.fill_diagonal(N1, 2.0)
    N1 *= dis1[:, None] * dis1[None, :]
    up = np.zeros_like(x_d2)
    up[perm2] = x_d3
    z2 = (x_d2 + up) @ np.asarray(W_up[0], np.float32)
    nc4b = build_k4b()
    rpc1 = 2048 // NCORES
    nt2_pm = _pm(np.ascontiguousarray(N2.T), BFNP)
    z2_pm = _pm(z2, BFNP)
    xds = dis1[:, None] * x_d1
    xdh = xds.astype(F8NP)
    xdl = (xds - xdh.astype(np.float32)).astype(F8NP)
    xdh_pm, xdl_pm = _pm(xdh, F8NP), _pm(xdl, F8NP)
    wu1_pm = _pm(np.asarray(W_up[1], np.float32), BFNP)
    ones = np.ones((1, 128), BFNP)
    b0 = np.asarray(b_up[0], np.float32)[None, :].astype(BFNP)
    b1v = np.asarray(b_up[1], np.float32)
    maps = []
    for cc in range(NCORES):
        sl = slice(cc * rpc1, (cc + 1) * rpc1)
        nt1r = np.ascontiguousarray(M1[sl].T)      # raw ints, diag := 2
        nt1r[cc * rpc1 + np.arange(rpc1), np.arange(rpc1)] = 2.0
        maps.append({
            "NT2": nt2_pm, "Z2": z2_pm,
            "XDH": xdh_pm, "XDL": xdl_pm,
            "NT1R": _pm(nt1r, F8NP),
            "Q1B": _pm(np.ascontiguousarray(
                (N1[sl][:, perm1] / dis1[sl][:, None]).T), BFNP),
            "WU1": wu1_pm, "ONES": ones, "B0": b0,
            "DISPQ": _pmv(dis1[sl]),
            "IVB1": _ivbv(1.0 / dis1[sl], b1v, C),
            })
    res = _run(nc4b, maps)
    xU2 = np.concatenate([_unpm(r["XO"]) for r in res], 0)

    # ---- K4c
    upf = np.zeros_like(x0)
    upf[perm0] = xU2
    zf = (x0 + upf) @ np.asarray(W_final, np.float32)
    nc4c = build_diag()
    maps = _diag_inputs(A8T, dis0[:, None] * zf, dis0,
                        np.asarray(b_final, np.float32), rpc0)
    res = _run(nc4c, maps)
    out = np.concatenate([_unpm(r["XO"]) for r in res], 0)
    return out.astype(np.float32)
